# revision 1
# baseline (speedup 1.0000x reference)
"""Trainium2 Bass kernel for causal multi-head attention (dense transformer block).

Math (reference semantics):
    qkv = x @ w_qkv.T ; split into Q,K,V heads [B,H,T,dk]
    (rotary in the reference rotates Q and K of head h by a constant,
     time-independent orthogonal rotation R_h; since scores = (R_h q)·(R_h k)
     = q·k, the rotation cancels exactly and is skipped here)
    scores = causal_mask(Q @ K.T / sqrt(dk)); attn = softmax(scores)
    out = attn @ V ; y = out @ w_o.T

Sharding: head-parallel over 8 cores (2 heads/core, both batches).  Each core
computes a partial y (its heads' contribution through w_o columns); the host
sums the 8 partials (the "all-reduce").

All matmuls run as float32r (TF32-class precision, full PE rate at N>=256).
"""

import numpy as np

import concourse.bacc as bacc
import concourse.bass as bass
import concourse.mybir as mybir
import concourse.tile as tile
from concourse import bass_utils
from concourse.masks import make_identity

B, T, D, H, DK = 2, 2048, 2048, 16, 128
NCORES = 8
HPC = H // NCORES  # heads per core
P = 128
NB = 512  # free-dim block (phase1 token block, phase2 query block, phase3 e block)
KC = D // P  # 16 contraction chunks of the model dim
QB = T // NB  # 4 query blocks per batch
NT = T // P  # 16 token tiles / key tiles per batch
FP32 = mybir.dt.float32
F32R = mybir.dt.float32r
SCALE = 1.0 / np.sqrt(DK)


DEFAULT_OPTS = dict(share_vt=False, xt2=4, skew=2, ps_alt=False, qkv_copy="vector",
                    v_copy="any", yt_copy="alt", ex_bufs=3, pss_bufs=4, mask_window=True,
                    persist=True, yt_bufs=4, bc_bufs=1, rec_bufs=1, psd_bufs=1,
                    dma_order=True, ph3_cycle=False, bc_copy="vector", w_by_m=False,
                    py_bufs=2, x_ring="sync", y_ring="sync", qb_outer=True, loop_n=1, phases=123, ilv_h=False, mask_mode="dve", exp2=False, po_bufs=1)


def build(debug=False, **opts):
    o = dict(DEFAULT_OPTS); o.update(opts)
    nc = bacc.Bacc("TRN2", target_bir_lowering=False, debug=False, num_devices=NCORES)
    xT = nc.dram_tensor("xT", [D, B * T], F32R, kind="ExternalInput")
    if o["w_by_m"]:
        wqkvT = nc.dram_tensor("wqkvT", [6, D, DK], F32R, kind="ExternalInput")
    else:
        wqkvT = nc.dram_tensor("wqkvT", [D, 6 * DK], F32R, kind="ExternalInput")
    woT = nc.dram_tensor("woT", [HPC * DK, D], F32R, kind="ExternalInput")
    masks = nc.dram_tensor("masks", [4, P, NB], F32R, kind="ExternalInput")
    ones_col = nc.dram_tensor("ones_col", [P, 1], F32R, kind="ExternalInput")
    ones_row = nc.dram_tensor("ones_row", [1, P], F32R, kind="ExternalInput")
    y = nc.dram_tensor("y", [B * T, D], FP32, kind="ExternalOutput")
    dbg = {}
    if debug:
        dbg["qkvT"] = nc.dram_tensor("dbg_qkvT", [6 * DK, B * T], F32R, kind="ExternalOutput")
        dbg["outT"] = nc.dram_tensor("dbg_outT", [HPC * DK, B * T], F32R, kind="ExternalOutput")

    with tile.TileContext(nc) as tc:
        with (
            tc.tile_pool(name="const", bufs=1) as cpool,
            tc.tile_pool(name="xp", bufs=2) as xpool,
            tc.tile_pool(name="qkv", bufs=1) as qpool,
            tc.tile_pool(name="attn", bufs=1) as apool,
            tc.tile_pool(name="ps", bufs=1, space="PSUM") as pspool,
        ):
            # ---- constants / weights resident in SBUF ----
            w_sb = [cpool.tile([P, 6 * DK], F32R, name=f"w_{k}") for k in range(KC)]
            wo_sb = [cpool.tile([P, D], F32R, name=f"wo_{h}") for h in range(HPC)]
            def load_w(m, k):
                if o["w_by_m"]:
                    nc.sync.dma_start(w_sb[k][:, m * P:(m + 1) * P],
                                      wqkvT[m, k * P:(k + 1) * P, :])
                else:
                    nc.sync.dma_start(w_sb[k][:], wqkvT[k * P:(k + 1) * P, :])

            if not o["dma_order"]:
                if o["w_by_m"]:
                    for m in range(6):
                        for k in range(KC):
                            load_w(m, k)
                else:
                    for k in range(KC):
                        load_w(0, k)
                for h in range(HPC):
                    nc.sync.dma_start(wo_sb[h][:], woT[h * P:(h + 1) * P, :])
            def load_small():
                if o["mask_window"]:
                    nc.sync.dma_start(maskw[:, 0:NB], masks[3])
                    nc.sync.dma_start(maskw[:, NB:NB + P], masks[0][:, NB - 3 * P:NB - 2 * P])
                    nc.sync.dma_start(maskw[:, NB + P:NB + 2 * P], masks[0][:, NB - 2 * P:NB - P])
                    nc.sync.dma_start(maskw[:, NB + 2 * P:NB + 3 * P], masks[0][:, NB - P:NB])
                else:
                    for j in range(4):
                        nc.sync.dma_start(mask_sb[j][:], masks[j])
                nc.sync.dma_start(onc[:], ones_col[:, :])
                nc.sync.dma_start(onr[:], ones_row[:, :])

            if o["mask_window"]:
                maskw = cpool.tile([P, NB + 3 * P], F32R, name="maskw")
                mask_sb = [maskw[:, 3 * P - j * P: 3 * P - j * P + NB] for j in range(4)]
            else:
                mask_sb = [cpool.tile([P, NB], F32R, name=f"mask_{j}") for j in range(4)]
            onc = cpool.tile([P, 1], F32R, name="onc")
            onr = cpool.tile([1, P], F32R, name="onr")
            ident = cpool.tile([P, P], FP32, name="ident")
            make_identity(nc, ident[:])
            if not o["dma_order"]:
                load_small()

            # alternate PSUM tags so back-to-back groups double-buffer across
            # the two "spare" tags without exceeding the 8-bank budget
            def ps_alt(i, shape, name):
                if o["ps_alt"]:
                    return pspool.tile(shape, FP32, name=name,
                                       tag="ps_y" if i % 2 == 0 else "ps_o",
                                       bufs=o["py_bufs"] if i % 2 == 0 else 2)
                return pspool.tile(shape, FP32, name=name, tag="ps_y", bufs=o["py_bufs"])

            if o["persist"]:
                QTp = [qpool.tile([P, T], F32R, name=f"QT{h}") for h in range(HPC)]
                KTp = [qpool.tile([P, T], F32R, name=f"KT{h}") for h in range(HPC)]
                VTp = [qpool.tile([P, T], FP32, name=f"VT{h}") for h in range(HPC)]
                Vp = [[qpool.tile([P, DK], F32R, name=f"V{h}_{kt}") for kt in range(NT)]
                      for h in range(HPC)]
                outTp = [qpool.tile([P, T], F32R, name=f"outT{h}") for h in range(HPC)]

            import contextlib
            loop_ctx = (tc.For_i(0, o["loop_n"], 1, hint_engines=(mybir.EngineType.PE,
                        mybir.EngineType.Activation, mybir.EngineType.DVE,
                        mybir.EngineType.SP))
                        if o["loop_n"] > 1 else contextlib.nullcontext())
            with loop_ctx:
              for b in range(B):
                  # per-batch tiles; tags make slots recycle across batches.
                  # VT shares slots with outT (disjoint lifetimes within a batch).
                  QT = QTp if o["persist"] else [qpool.tile([P, T], F32R, name=f"QT{h}_{b}", tag=f"QT{h}") for h in range(HPC)]
                  KT = KTp if o["persist"] else [qpool.tile([P, T], F32R, name=f"KT{h}_{b}", tag=f"KT{h}") for h in range(HPC)]
                  VT = VTp if o["persist"] else [qpool.tile([P, T], FP32, name=f"VT{h}_{b}", tag=(f"vo{h}" if o["share_vt"] else f"VT{h}")) for h in range(HPC)]
                  V = Vp if o["persist"] else [[qpool.tile([P, DK], F32R, name=f"V{h}_{kt}_{b}", tag=f"V{h}_{kt}")
                        for kt in range(NT)] for h in range(HPC)]
                  mdest = [QT[0], KT[0], VT[0], QT[1], KT[1], VT[1]]

                  # ======== phase 1: QKV^T = wqkvT.T @ xT_block ========
                  for nb in range(QB):
                      xt = [xpool.tile([P, NB], F32R, name=f"x{k}_{b}_{nb}", tag=f"x{k}",
                                       bufs=(2 if k < o["xt2"] else 1))
                            for k in range(KC)]
                      col0 = b * T + nb * NB
                      for k in range(KC):
                          if o["dma_order"] and b == 0 and nb == 0:
                              load_w(0, k)
                          getattr(nc, o["x_ring"]).dma_start(xt[k][:], xT[k * P:(k + 1) * P, col0:col0 + NB])
                      if o["dma_order"] and b == 0 and nb == 0:
                          load_small()
                          if o["w_by_m"]:
                              for m in range(1, 6):
                                  for k in range(KC):
                                      load_w(m, k)
                      if o["dma_order"] and b == 0 and nb == 1:
                          for h in range(HPC):
                              nc.sync.dma_start(wo_sb[h][:], woT[h * P:(h + 1) * P, :])
                      for m in range(6):
                          ps = pspool.tile([P, NB], FP32, name=f"ps1_{b}_{nb}_{m}",
                                           tag="ps_s", bufs=o["pss_bufs"])
                          for k in range(KC):
                              nc.tensor.matmul(ps[:], w_sb[k][:, m * P:(m + 1) * P],
                                               xt[k][:], start=(k == 0), stop=(k == KC - 1))
                          getattr(nc, o["qkv_copy"]).tensor_copy(mdest[m][:, nb * NB:(nb + 1) * NB], ps[:]) if o["qkv_copy"] != "scalar" else nc.scalar.copy(mdest[m][:, nb * NB:(nb + 1) * NB], ps[:])
                  # V^T -> V (PE transpose per 128x128 tile)
                  for h in range(HPC if o["phases"] != 1 else 0):
                      for kt in range(NT):
                          pst = ps_alt(kt, [P, P], f"pst_{b}_{h}_{kt}")
                          nc.tensor.transpose(pst[:], VT[h][:, kt * P:(kt + 1) * P], ident[:])
                          getattr(nc, o["v_copy"]).tensor_copy(V[h][kt][:], pst[:])

                  if debug:
                      for m in range(6):
                          nc.sync.dma_start(
                              dbg["qkvT"][m * P:(m + 1) * P, b * T:(b + 1) * T], mdest[m][:])

                  # ======== phase 2: attention per head ========
                  outT = outTp if o["persist"] else [qpool.tile([P, T], F32R, name=f"outT{h}_{b}",
                                     tag=(f"vo{h}" if o["share_vt"] else f"oT{h}"))
                          for h in range(HPC)]
                  def attention_gen(h, qb, b=b, QT=QT, KT=KT, V=V, outT=outT):
                      nkt = 4 * qb + 4  # causal: key tiles 0..nkt-1
                      ps_o = pspool.tile([P, NB], FP32, name=f"pso_{b}_{h}_{qb}",
                                         tag="ps_o", bufs=o["po_bufs"])
                      ps_d = pspool.tile([1, NB], FP32, name=f"psd_{b}_{h}_{qb}",
                                         tag="ps_d", bufs=o["psd_bufs"])
                      qsl = slice(qb * NB, (qb + 1) * NB)

                      def scores(kt):
                          ps_s = pspool.tile([P, NB], FP32, name=f"pss_{b}_{h}_{qb}_{kt}",
                                             tag="ps_s", bufs=o["pss_bufs"])
                          nc.tensor.matmul(ps_s[:], KT[h][:, kt * P:(kt + 1) * P],
                                           QT[h][:, qsl], start=True, stop=True)
                          return ps_s

                      if o["exp2"]:
                          # paired kt steps: one 2-bank scores tile + one FD=1024 exp
                          for kp_ in range(nkt // 2):
                              kt0, kt1 = 2 * kp_, 2 * kp_ + 1
                              ps2 = pspool.tile([P, 2 * NB], FP32, name=f"pss2_{b}_{h}_{qb}_{kp_}",
                                                tag="ps_s", bufs=2)
                              nc.tensor.matmul(ps2[:, 0:NB], KT[h][:, kt0 * P:(kt0 + 1) * P],
                                               QT[h][:, qsl], start=True, stop=True)
                              nc.tensor.matmul(ps2[:, NB:2 * NB], KT[h][:, kt1 * P:(kt1 + 1) * P],
                                               QT[h][:, qsl], start=True, stop=True)
                              ex2 = apool.tile([P, 2 * NB], F32R, name=f"ex2_{b}_{h}_{qb}_{kp_}",
                                               tag="ex", bufs=2)
                              nc.scalar.activation(ex2[:], ps2[:],
                                                   mybir.ActivationFunctionType.Exp,
                                                   scale=SCALE)
                              for kt, exv in ((kt0, ex2[:, 0:NB]), (kt1, ex2[:, NB:2 * NB])):
                                  j = kt - 4 * qb
                                  if j >= 0:
                                      nc.vector.tensor_mul(exv, exv, mask_sb[j][:])
                                  nc.tensor.matmul(ps_d[:1, :], onc[:, :], exv,
                                                   start=(kt == 0), stop=(kt == nkt - 1),
                                                   skip_group_check=True)
                                  nc.tensor.matmul(ps_o[:], V[h][kt][:], exv,
                                                   start=(kt == 0), stop=(kt == nkt - 1),
                                                   skip_group_check=True)
                              yield
                          # skip the plain path
                          nkt = 0
                      pss = {}
                      for i in range(min(o["skew"], nkt)):
                          pss[i] = scores(i)
                      for kt in range(nkt):
                          yield
                          ps_s = pss.pop(kt)
                          ex = apool.tile([P, NB], F32R, name=f"ex_{b}_{h}_{qb}_{kt}",
                                          tag="ex", bufs=o["ex_bufs"])
                          nc.scalar.activation(ex[:], ps_s[:],
                                               mybir.ActivationFunctionType.Exp,
                                               scale=SCALE)
                          j = kt - 4 * qb
                          if j >= 0:
                              if o["mask_mode"] == "affine":
                                  nc.gpsimd.affine_select(
                                      ex[:], ex[:], pattern=[[1, NB]],
                                      compare_op=mybir.AluOpType.is_ge,
                                      fill=0.0, base=-128 * j, channel_multiplier=-1)
                              else:
                                  nc.vector.tensor_mul(ex[:], ex[:], mask_sb[j][:])
                          if kt + o["skew"] < nkt:
                              pss[kt + o["skew"]] = scores(kt + o["skew"])
                          nc.tensor.matmul(ps_d[:1, :], onc[:, :], ex[:],
                                           start=(kt == 0), stop=(kt == nkt - 1),
                                           skip_group_check=True)
                          nc.tensor.matmul(ps_o[:], V[h][kt][:], ex[:],
                                           start=(kt == 0), stop=(kt == nkt - 1),
                                           skip_group_check=True)
                      # normalize: outT[:, qsl] = ps_o * (1/ps_d) broadcast over partitions
                      rec = apool.tile([1, NB], F32R, name=f"rec_{b}_{h}_{qb}",
                                       tag="rec", bufs=o["rec_bufs"])
                      with nc.allow_low_precision(reason="f32r recip: tf32 rounding intended"):
                          nc.vector.reciprocal(rec[:1, :], ps_d[:1, :])
                      ps_b = pspool.tile([P, NB], FP32, name=f"psb_{b}_{h}_{qb}",
                                         tag="ps_s", bufs=o["pss_bufs"])
                      nc.tensor.matmul(ps_b[:], onr[:1, :], rec[:1, :],
                                       start=True, stop=True)
                      bc = apool.tile([P, NB], FP32, name=f"bc_{b}_{h}_{qb}",
                                      tag="bc", bufs=o["bc_bufs"])
                      nc.vector.tensor_copy(bc[:], ps_b[:]) if o["bc_copy"] == "vector" else nc.scalar.copy(bc[:], ps_b[:])
                      nc.vector.tensor_mul(outT[h][:, qsl], ps_o[:], bc[:])
                      yield

                  def attention(h, qb):
                      for _ in attention_gen(h, qb):
                          pass

                  def attention_pair(qb):
                      gens = [attention_gen(0, qb), attention_gen(1, qb)]
                      done = [False, False]
                      while not all(done):
                          for gi in range(2):
                              if not done[gi]:
                                  try:
                                      next(gens[gi])
                                  except StopIteration:
                                      done[gi] = True

                  def proj(tt, eb, b=b, outT=outT):
                      i3 = tt * QB + eb
                      ps = ps_alt(i3, [P, NB], f"psy_{b}_{tt}_{eb}")
                      for h in range(HPC):
                          nc.tensor.matmul(ps[:], outT[h][:, tt * P:(tt + 1) * P],
                                           wo_sb[h][:, eb * NB:(eb + 1) * NB],
                                           start=(h == 0), stop=(h == HPC - 1))
                      yt = apool.tile([P, NB], FP32, name=f"yt_{b}_{tt}_{eb}",
                                      tag="yt", bufs=o["yt_bufs"])
                      if o["yt_copy"] == "alt":
                          eng = nc.vector if i3 % 2 == 0 else nc.scalar
                          eng.tensor_copy(yt[:], ps[:]) if eng is nc.vector else eng.copy(yt[:], ps[:])
                      else:
                          getattr(nc, o["yt_copy"]).tensor_copy(yt[:], ps[:])
                      getattr(nc, o["y_ring"]).dma_start(
                          y[b * T + tt * P: b * T + (tt + 1) * P, eb * NB:(eb + 1) * NB],
                          yt[:])

                  if o["phases"] == 1:
                      # dump QT directly so phase1 outputs are live
                      for m in [0]:
                          yt0 = apool.tile([P, NB], FP32, name=f"p1y_{b}", tag="yt", bufs=o["yt_bufs"])
                          nc.vector.tensor_copy(yt0[:], QT[0][:, 0:NB])
                          nc.sync.dma_start(y[b * T: b * T + P, 0:NB], yt0[:])
                  elif o["phases"] == 12:
                      for qb in range(QB):
                          for h in range(HPC):
                              attention(h, qb)
                      for h in range(HPC):
                          yth = apool.tile([P, NB], FP32, name=f"p2y_{b}_{h}", tag="yt", bufs=o["yt_bufs"])
                          nc.vector.tensor_copy(yth[:], outT[h][:, 0:NB])
                          nc.sync.dma_start(y[b * T + h * P: b * T + (h + 1) * P, 0:NB], yth[:])
                  elif o["qb_outer"]:
                      for qb in range(QB):
                          if o["ilv_h"]:
                              attention_pair(qb)
                          else:
                              for h in range(HPC):
                                  attention(h, qb)
                          for tt in range(4 * qb, 4 * qb + 4):
                              for eb in range(QB):
                                  proj(tt, eb)
                  else:
                      for h in range(HPC):
                          for qb in range(QB):
                              attention(h, qb)
                      for tt in range(NT):
                          for eb in range(QB):
                              proj(tt, eb)

                  if debug:
                      for h in range(HPC):
                          nc.sync.dma_start(
                              dbg["outT"][h * P:(h + 1) * P, b * T:(b + 1) * T], outT[h][:])

    nc.compile()
    return nc


def prep_inputs(x, w_qkv, w_o, w_by_m=False):
    """Host-side shard prep. Returns per-core input maps."""
    x = np.ascontiguousarray(np.asarray(x, dtype=np.float32).reshape(B * T, D).T)
    w_qkv = np.asarray(w_qkv, dtype=np.float32)
    w_o = np.asarray(w_o, dtype=np.float32)

    mask = np.zeros((4, P, NB), dtype=np.float32)
    for j in range(4):
        kp = np.arange(P)[:, None] + j * P
        qf = np.arange(NB)[None, :]
        mask[j] = (kp <= qf).astype(np.float32)

    ones_col = np.ones((P, 1), dtype=np.float32)
    ones_row = np.ones((1, P), dtype=np.float32)

    in_maps = []
    for c in range(NCORES):
        h0, h1 = HPC * c, HPC * c + 1
        blocks = []
        for h in (h0, h1):
            blocks += [w_qkv[h * DK:(h + 1) * DK],          # Q rows
                       w_qkv[D + h * DK: D + (h + 1) * DK],  # K rows
                       w_qkv[2 * D + h * DK: 2 * D + (h + 1) * DK]]  # V rows
        # order: [Qh0,Kh0,Vh0,Qh1,Kh1,Vh1]
        if w_by_m:
            wq = np.ascontiguousarray(np.stack([blk.T for blk in blocks]))  # [6, D, 128]
        else:
            wq = np.ascontiguousarray(np.concatenate(blocks, axis=0).T)  # [D, 768]
        wo = np.ascontiguousarray(w_o[:, HPC * DK * c: HPC * DK * (c + 1)].T)  # [256, D]
        in_maps.append({
            "xT": x, "wqkvT": wq, "woT": wo,
            "masks": mask, "ones_col": ones_col, "ones_row": ones_row,
        })
    return in_maps


_nc_cache = {}


def get_nc(debug=False, **opts):
    key = (debug, tuple(sorted(opts.items())))
    if key not in _nc_cache:
        _nc_cache[key] = build(debug=debug, **opts)
    return _nc_cache[key]


def run(x, w_qkv, w_o, debug=False, **opts):
    nc = get_nc(debug=debug, **opts)
    o = dict(DEFAULT_OPTS); o.update(opts)
    in_maps = prep_inputs(x, w_qkv, w_o, w_by_m=o["w_by_m"])
    res = bass_utils.run_bass_kernel_spmd(nc, in_maps, core_ids=list(range(NCORES)))
    return res


def kernel(x, w_qkv, w_o):
    res = run(x, w_qkv, w_o)
    y = res.results[0]["y"].astype(np.float64)
    for c in range(1, NCORES):
        y += res.results[c]["y"]
    return y.astype(np.float32).reshape(B, T, D)



# revision 15
# speedup vs baseline: 1.1600x; 1.1600x over previous
"""Trainium2 Bass kernel for causal multi-head attention (dense transformer block).

Math (reference semantics):
    qkv = x @ w_qkv.T ; split into Q,K,V heads [B,H,T,dk]
    (rotary in the reference rotates Q and K of head h by a constant,
     time-independent orthogonal rotation R_h; since scores = (R_h q)·(R_h k)
     = q·k, the rotation cancels exactly and is skipped here)
    scores = causal_mask(Q @ K.T / sqrt(dk)); attn = softmax(scores)
    out = attn @ V ; y = out @ w_o.T

Sharding: head-parallel over 8 cores (2 heads/core, both batches).  Each core
computes a partial y (its heads' contribution through w_o columns); the host
sums the 8 partials (the "all-reduce").

v3 design (vs the f32r baseline):
  * All matmul inputs bf16 (same 1 cyc/row PE rate as f32r at wide free dims,
    half the DMA + SBUF).  PSUM accumulation stays fp32.
  * Phase 1 is k-outer: 6 concurrent PSUM groups (Q/K for 2 heads in two
    2-bank "S" tiles, V written DIRECTLY in [token, dk] layout into "O"
    tiles) so PE starts as soon as the first weight/x chunk lands and no
    V^T->V transposes are needed.  PSUM drains on ACT/DVE (GpSimd cannot
    touch PSUM), chunk-PAIR DMAs halve HWDGE descriptor-queue pressure.
  * Softmax denominator: bf16 pair-add (DVE 4x mode) + fp32 running sums
    split into two chains (GpSimd + DVE), then two GpSimd cross-partition
    (axis=C) reduces - no [1,512] ones-matmuls on PE (saves ~34us PE).
  * Causal narrowing: diagonal key-tiles only compute the live q-suffix in
    scores/AV/exp; the dead ex prefix is zeroed by a GpSimd memset; the
    128x128 causal triangle is masked by a GpSimd multiply.
  * proj(qb) units are woven between the attention kt-pairs of the next
    unit (qb3 into the next batch's phase 1 + qb0 unit) so the exp-gated
    stretches of attention get PE filler; yt PSUM->SBUF copies alternate
    ACT/DVE; y stores go out as one wide DMA per token tile.
  * softmax close-out chains (reduce -> recip -> broadcast-mm -> normalize)
    are deferred into the following instruction stream so PE (in-order)
    never waits on them.
"""

import contextlib

import numpy as np

import concourse.bacc as bacc
import concourse.bass as bass
import concourse.mybir as mybir
import concourse.tile as tile
from concourse import bass_utils

B, T, D, H, DK = 2, 2048, 2048, 16, 128
NCORES = 8
HPC = H // NCORES  # heads per core
P = 128
NB = 512           # q-block / token-block / e-block width
KC = D // P        # 16 contraction chunks of the model dim
KP = KC // 2       # chunk pairs
QB = T // NB       # 4 q blocks per batch
NT = T // P        # 16 token tiles per batch
WC = 6 * P         # w columns per chunk: Q0 K0 Q1 K1 V0 V1
FP32 = mybir.dt.float32
F32R = mybir.dt.float32r
BF16 = mybir.dt.bfloat16
SCALE = 1.0 / np.sqrt(DK)

DEFAULT_OPTS = dict(
    ex_bufs=3, xt_bufs=2, yt_bufs=2, s_bufs=2, o_bufs=2, y_bufs=2,
    loop_n=1,
    # yt-copy engine cycle per *hosting location* of the proj units
    pat_attn={0: "DA", 1: "DA", 2: "DA", 3: "DA"},
    pat_p1="DA", pat_tail="DA",
    defer_pairs=2,
    chain_pat="PD",       # exsum chain engines (even pair, odd pair)
    mask_eng="P",
    qkv_q="A", qkv_k="D", qkv_v="AD",
)

_ENG_MAP = {"P": "gpsimd", "A": "scalar", "D": "vector"}


def _width(kt, qb):
    """Live q-suffix width of key tile kt within q-block qb (causal)."""
    j = kt - 4 * qb
    if j <= 0:
        return NB
    return NB - P * j


def build(debug=False, **opts):
    o = dict(DEFAULT_OPTS)
    o.update({k: v for k, v in opts.items() if k in DEFAULT_OPTS})
    nc = bacc.Bacc("TRN2", target_bir_lowering=False, debug=False,
                   num_devices=NCORES)
    # 3D dram layouts allow one DMA per chunk-pair / token tile
    xT = nc.dram_tensor("xT", [KC, P, B * T], BF16, kind="ExternalInput")
    w_d = nc.dram_tensor("w", [KC, P, WC], BF16, kind="ExternalInput")
    woT = nc.dram_tensor("woT", [HPC * DK, D], BF16, kind="ExternalInput")
    tri_d = nc.dram_tensor("tri", [P, P], BF16, kind="ExternalInput")
    onr_d = nc.dram_tensor("onr", [1, P], F32R, kind="ExternalInput")
    y = nc.dram_tensor("y", [B * T, D], FP32, kind="ExternalOutput")

    with tile.TileContext(nc) as tc:
        with (
            tc.tile_pool(name="const", bufs=1) as cpool,
            tc.tile_pool(name="xp", bufs=1) as xpool,
            tc.tile_pool(name="qkv", bufs=1) as qpool,
            tc.tile_pool(name="attn", bufs=1) as apool,
            tc.tile_pool(name="ps", bufs=1, space="PSUM") as pspool,
        ):
            # ---- constants / weights resident in SBUF ----
            w_sb = [cpool.tile([P, 2, WC], BF16, name=f"w_{kp}") for kp in range(KP)]
            wo_sb = [cpool.tile([P, D], BF16, name=f"wo_{h}") for h in range(HPC)]
            tri = cpool.tile([P, P], BF16, name="tri")
            onr = cpool.tile([1, P], F32R, name="onr")

            def wqk(k, m):  # m in 0..3 = Q0 K0 Q1 K1 of chunk k
                return w_sb[k // 2][:, k % 2, m * P:(m + 1) * P]

            def wvv(k):     # V columns (both heads) of chunk k
                return w_sb[k // 2][:, k % 2, 4 * P:6 * P]

            # persistent per-batch state (WAR deps recycle across batches)
            QTp = [qpool.tile([P, T], BF16, name=f"QT{h}") for h in range(HPC)]
            KTp = [qpool.tile([P, T], BF16, name=f"KT{h}") for h in range(HPC)]
            Vp = [qpool.tile([P, 2 * P], BF16, name=f"V{kt}") for kt in range(NT)]
            outTp = [qpool.tile([P, T], BF16, name=f"outT{h}") for h in range(HPC)]

            def ps_tile(tag, shape, name, bufs):
                return pspool.tile(shape, FP32, name=name, tag=tag, bufs=bufs)

            def copy_on(code, dst, src):
                eng = _ENG_MAP[code]
                if eng == "gpsimd":
                    nc.gpsimd.tensor_copy(dst, src)
                elif eng == "scalar":
                    nc.scalar.copy(dst, src)
                else:
                    nc.vector.tensor_copy(dst, src)

            loop_ctx = (tc.For_i(0, o["loop_n"], 1, hint_engines=(
                            mybir.EngineType.PE, mybir.EngineType.Activation,
                            mybir.EngineType.DVE, mybir.EngineType.SP,
                            mybir.EngineType.Pool))
                        if o["loop_n"] > 1 else contextlib.nullcontext())

            if o["loop_n"] > 1:
                # weights/constants loaded once, outside the HW loop
                for kp in range(KP):
                    nc.sync.dma_start(w_sb[kp][:], w_d[2 * kp:2 * kp + 2])
                nc.sync.dma_start(tri[:], tri_d[:, :])
                nc.sync.dma_start(onr[:], onr_d[:, :])
                for h in range(HPC):
                    nc.sync.dma_start(wo_sb[h][:], woT[h * P:(h + 1) * P, :])

            with loop_ctx:
                # ============ phase 1 generator (one token block) ============
                def phase1_nb(b, nb):
                    """QKV projection for token block (b, nb), k-outer.
                    Yields after each chunk-pair (8) + once at the drain."""
                    col0 = b * T + nb * NB
                    xt = [xpool.tile([P, 2, NB], BF16, name=f"x{kp}_{b}_{nb}",
                                     tag=f"x{kp}", bufs=o["xt_bufs"])
                          for kp in range(KP)]
                    first = b == 0 and nb == 0 and o["loop_n"] == 1
                    for kp in range(KP):
                        if first:
                            nc.sync.dma_start(w_sb[kp][:], w_d[2 * kp:2 * kp + 2])
                        nc.sync.dma_start(xt[kp][:],
                                          xT[2 * kp:2 * kp + 2, :, col0:col0 + NB])
                        if first and kp == 0:
                            nc.sync.dma_start(tri[:], tri_d[:, :])
                            nc.sync.dma_start(onr[:], onr_d[:, :])
                    if b == 0 and nb == 1 and o["loop_n"] == 1:
                        for h in range(HPC):
                            nc.sync.dma_start(wo_sb[h][:], woT[h * P:(h + 1) * P, :])

                    S0 = ps_tile("S", [P, 2, NB], f"p1s0_{b}_{nb}", o["s_bufs"])
                    S1 = ps_tile("S", [P, 2, NB], f"p1s1_{b}_{nb}", o["s_bufs"])
                    # V token-tile groups need a PSUM bank each (one
                    # accumulation group per bank): two sub-sweeps of 2.
                    V01 = [ps_tile("O", [P, NB], f"p1v{t}_{b}_{nb}", o["o_bufs"])
                           for t in range(2)]
                    for kp in range(KP):
                        for half in range(2):
                            k = 2 * kp + half
                            st, sp = k == 0, k == KC - 1
                            xk = xt[kp][:, half, :]
                            nc.tensor.matmul(S0[:, 0, :], wqk(k, 0), xk,
                                             start=st, stop=sp)
                            nc.tensor.matmul(S0[:, 1, :], wqk(k, 1), xk,
                                             start=st, stop=sp)
                            nc.tensor.matmul(S1[:, 0, :], wqk(k, 2), xk,
                                             start=st, stop=sp)
                            nc.tensor.matmul(S1[:, 1, :], wqk(k, 3), xk,
                                             start=st, stop=sp)
                            for t in range(2):
                                nc.tensor.matmul(V01[t][:, 0:2 * P],
                                                 xt[kp][:, half, t * P:(t + 1) * P],
                                                 wvv(k), start=st, stop=sp)
                        yield
                    # drain QK + first V pair; second V pair sweep follows
                    csl = slice(nb * NB, (nb + 1) * NB)
                    copy_on(o["qkv_q"], QTp[0][:, csl], S0[:, 0, :])
                    copy_on(o["qkv_k"], KTp[0][:, csl], S0[:, 1, :])
                    copy_on(o["qkv_q"], QTp[1][:, csl], S1[:, 0, :])
                    copy_on(o["qkv_k"], KTp[1][:, csl], S1[:, 1, :])
                    for t in range(2):
                        copy_on(o["qkv_v"][t % len(o["qkv_v"])],
                                Vp[nb * 4 + t][:], V01[t][:, 0:2 * P])
                    yield
                    V23 = [ps_tile("O", [P, NB], f"p1v{2 + t}_{b}_{nb}",
                                   o["o_bufs"]) for t in range(2)]
                    for kp in range(KP):
                        for half in range(2):
                            k = 2 * kp + half
                            st, sp = k == 0, k == KC - 1
                            for t in range(2):
                                nc.tensor.matmul(V23[t][:, 0:2 * P],
                                                 xt[kp][:, half,
                                                        (2 + t) * P:(3 + t) * P],
                                                 wvv(k), start=st, stop=sp)
                        if kp % 2 == 1:
                            yield
                    for t in range(2):
                        copy_on(o["qkv_v"][t % len(o["qkv_v"])],
                                Vp[nb * 4 + 2 + t][:], V23[t][:, 0:2 * P])
                    yield

                # ============ attention generator (one head) ============
                def attention_gen(b, h, qb):
                    """Yields once per kt-pair.  Returns the deferred
                    close-out thunk (bcmm + normalize)."""
                    nkt = 4 * qb + 4
                    qsl0 = qb * NB
                    ps_o = ps_tile("O", [P, NB], f"pso_{b}_{h}_{qb}", o["o_bufs"])
                    # two fp32 running-sum chains: even pairs / odd pairs
                    exs = [apool.tile([P, NB], FP32, name=f"exs{i}_{b}_{h}_{qb}",
                                      tag=f"exsum{i}", bufs=2) for i in range(2)]
                    npair = nkt // 2
                    for p in range(npair):
                        a, c = 2 * p, 2 * p + 1
                        wa, wc = _width(a, qb), _width(c, qb)
                        oa, oc = NB - wa, NB - wc
                        om = min(oa, oc)
                        ps_s = ps_tile("S", [P, 2, NB], f"pss_{b}_{h}_{qb}_{p}",
                                       o["s_bufs"])
                        nc.tensor.matmul(ps_s[:, 0, oa:NB],
                                         KTp[h][:, a * P:(a + 1) * P],
                                         QTp[h][:, qsl0 + oa:qsl0 + NB],
                                         start=True, stop=True)
                        nc.tensor.matmul(ps_s[:, 1, oc:NB],
                                         KTp[h][:, c * P:(c + 1) * P],
                                         QTp[h][:, qsl0 + oc:qsl0 + NB],
                                         start=True, stop=True)
                        ex = apool.tile([P, 2, NB], BF16,
                                        name=f"ex_{b}_{h}_{qb}_{p}",
                                        tag="ex", bufs=o["ex_bufs"])
                        if oa == oc:
                            nc.scalar.activation(ex[:, :, oa:NB],
                                                 ps_s[:, :, oa:NB],
                                                 mybir.ActivationFunctionType.Exp,
                                                 scale=SCALE)
                        else:
                            nc.scalar.activation(ex[:, 0, oa:NB],
                                                 ps_s[:, 0, oa:NB],
                                                 mybir.ActivationFunctionType.Exp,
                                                 scale=SCALE)
                            nc.scalar.activation(ex[:, 1, oc:NB],
                                                 ps_s[:, 1, oc:NB],
                                                 mybir.ActivationFunctionType.Exp,
                                                 scale=SCALE)
                        # zero dead prefixes of narrowed (diagonal) tiles
                        if oa > 0:
                            nc.gpsimd.memset(ex[:, 0, 0:oa], 0.0)
                        if oc > 0:
                            nc.gpsimd.memset(ex[:, 1, 0:oc], 0.0)
                        # triangle masks on diagonal tiles
                        for half, kt, off in ((0, a, oa), (1, c, oc)):
                            if kt >= 4 * qb:
                                sl = ex[:, half, off:off + P]
                                if o["mask_eng"] == "P":
                                    nc.gpsimd.tensor_mul(sl, sl, tri[:])
                                else:
                                    nc.vector.tensor_mul(sl, sl, tri[:])
                        # denominator partial: exs[p%2] += ex.lo + ex.hi
                        tpr = apool.tile([P, NB], BF16,
                                         name=f"tp_{b}_{h}_{qb}_{p}",
                                         tag="tpr", bufs=2)
                        nc.vector.tensor_add(tpr[:], ex[:, 0, :], ex[:, 1, :])
                        ce = _ENG_MAP[o["chain_pat"][p % 2]]
                        eng = getattr(nc, ce)
                        if p < 2:
                            eng.tensor_copy(exs[p % 2][:], tpr[:])
                        else:
                            eng.tensor_add(exs[p % 2][:], exs[p % 2][:], tpr[:])
                        # AV accumulation
                        nc.tensor.matmul(ps_o[:, oa:NB],
                                         Vp[a][:, h * P:(h + 1) * P],
                                         ex[:, 0, oa:NB],
                                         start=(p == 0), stop=False,
                                         skip_group_check=True)
                        nc.tensor.matmul(ps_o[:, oc:NB],
                                         Vp[c][:, h * P:(h + 1) * P],
                                         ex[:, 1, oc:NB],
                                         start=False, stop=(p == npair - 1),
                                         skip_group_check=True)
                        yield
                    # denominator: cross-partition reduces + merge + recip
                    dn = [apool.tile([1, NB], FP32, name=f"dn{i}_{b}_{h}_{qb}",
                                     tag=f"dn{i}", bufs=2) for i in range(2)]
                    rec = apool.tile([1, NB], F32R, name=f"rec_{b}_{h}_{qb}",
                                     tag="rec", bufs=2)
                    nc.gpsimd.tensor_reduce(dn[0][:1, :], exs[0][:],
                                            axis=mybir.AxisListType.C,
                                            op=mybir.AluOpType.add)
                    if npair > 1:
                        nc.gpsimd.tensor_reduce(dn[1][:1, :], exs[1][:],
                                                axis=mybir.AxisListType.C,
                                                op=mybir.AluOpType.add)
                        nc.vector.tensor_add(dn[0][:1, :], dn[0][:1, :],
                                             dn[1][:1, :])
                    with nc.allow_low_precision(reason="f32r recip: tf32 ok"):
                        nc.vector.reciprocal(rec[:1, :], dn[0][:1, :])

                    def close():
                        ps_bc = ps_tile("Y", [P, NB], f"psbc_{b}_{h}_{qb}",
                                        o["y_bufs"])
                        nc.tensor.matmul(ps_bc[:], onr[:1, :], rec[:1, :],
                                         start=True, stop=True)
                        bc = apool.tile([P, NB], FP32, name=f"bc_{b}_{h}_{qb}",
                                        tag="bc", bufs=2)
                        nc.vector.tensor_copy(bc[:], ps_bc[:])
                        nc.vector.tensor_mul(outTp[h][:, qsl0:qsl0 + NB],
                                             ps_o[:], bc[:])
                    return close

                # ============ proj units ============
                yts = {}

                def proj_unit(b, tt, eb, eng_code):
                    ps_y = ps_tile("Y", [P, NB], f"psy_{b}_{tt}_{eb}", o["y_bufs"])
                    nc.tensor.matmul(ps_y[:], outTp[0][:, tt * P:(tt + 1) * P],
                                     wo_sb[0][:, eb * NB:(eb + 1) * NB],
                                     start=True, stop=False)
                    nc.tensor.matmul(ps_y[:], outTp[1][:, tt * P:(tt + 1) * P],
                                     wo_sb[1][:, eb * NB:(eb + 1) * NB],
                                     start=False, stop=True)
                    if eb == 0:
                        yts[(b, tt)] = apool.tile([P, QB, NB], FP32,
                                                  name=f"yt_{b}_{tt}",
                                                  tag="yt", bufs=o["yt_bufs"])
                    yt = yts[(b, tt)]
                    copy_on(eng_code, yt[:, eb, :], ps_y[:])
                    if eb == QB - 1:
                        nc.sync.dma_start(
                            y[b * T + tt * P:b * T + (tt + 1) * P, :], yt[:])

                def proj_thunks(b, pqb, pat):
                    th = []
                    i = 0
                    for tt in range(4 * pqb, 4 * pqb + 4):
                        for eb in range(QB):
                            code = pat[i % len(pat)]
                            th.append(lambda b=b, tt=tt, eb=eb, code=code:
                                      proj_unit(b, tt, eb, code))
                            i += 1
                    return th

                # ============ weaving driver ============
                def weave(gen, fillers, carry, defer=None):
                    """Run gen; after each yield emit carry thunks (once,
                    after o['defer_pairs'] yields) and a fair share of
                    fillers (popped from the shared list)."""
                    n = 0
                    held = 0
                    try:
                        while True:
                            next(gen)
                            n += 1
                            if n >= o["defer_pairs"] and carry:
                                for fn in carry:
                                    fn()
                                carry = []
                            if defer:
                                held += defer.pop(0)
                            if not carry:
                                while held > 0 and fillers:
                                    fillers.pop(0)()
                                    held -= 1
                    except StopIteration as si:
                        for fn in carry:
                            fn()
                        return si.value

                def share(nfill, nsteps):
                    base, rem = divmod(nfill, nsteps)
                    return [base + (1 if i < rem else 0) for i in range(nsteps)]

                def attention_unit(b, qb, fillers, carry, reserve=3):
                    npair = 2 * qb + 2
                    # hold a few fillers back to cover the close-out chain
                    # latency after the last AV pair
                    nres = min(reserve, len(fillers))
                    sh = share(len(fillers) - nres, 2 * npair)
                    close0 = weave(attention_gen(b, 0, qb), fillers, carry,
                                   defer=sh[:npair])
                    close1 = weave(attention_gen(b, 1, qb), fillers, [close0],
                                   defer=sh[npair:])
                    for fn in fillers:  # reserved + leftovers
                        fn()
                    del fillers[:]
                    return [close1]

                # ============ main schedule ============
                carry = []
                for b in range(B):
                    # ---- phase 1 (+ second half of prev batch qb3 proj) ----
                    if b > 0:
                        ph1_fill = proj_thunks(b - 1, 3, o["pat_p1"])[8:]
                        qb0_fill = proj_thunks(b - 1, 3, o["pat_attn"][3])[:8]
                    else:
                        ph1_fill, qb0_fill = [], []
                    for nb in range(QB):
                        weave(phase1_nb(b, nb), ph1_fill, carry,
                              defer=share(2, KP + 1) if ph1_fill else None)
                        carry = []
                    for fn in ph1_fill:
                        fn()
                    # ---- attention + proj weave ----
                    carry = attention_unit(b, 0, qb0_fill, carry)
                    carry = attention_unit(
                        b, 1, proj_thunks(b, 0, o["pat_attn"][0]), carry)
                    carry = attention_unit(
                        b, 2, proj_thunks(b, 1, o["pat_attn"][1]), carry)
                    carry = attention_unit(
                        b, 3, proj_thunks(b, 2, o["pat_attn"][2]), carry)
                # ---- tail: close-out then final batch qb3 proj ----
                for c in carry:
                    c()
                carry = []
                for fn in proj_thunks(B - 1, 3, o["pat_tail"]):
                    fn()

    nc.compile()
    return nc


def prep_inputs(x, w_qkv, w_o):
    """Host-side shard prep. Returns per-core input maps (bf16)."""
    bf = mybir.dt.np(BF16)
    x = np.asarray(x, dtype=np.float32).reshape(B * T, D)
    xT = np.ascontiguousarray(x.T).reshape(KC, P, B * T).astype(bf)
    w_qkv = np.asarray(w_qkv, dtype=np.float32)
    w_o = np.asarray(w_o, dtype=np.float32)

    tri = np.zeros((P, P), dtype=np.float32)
    kp = np.arange(P)[:, None]
    qu = np.arange(P)[None, :]
    tri[kp <= qu] = 1.0
    tri = tri.astype(bf)
    onr = np.ones((1, P), dtype=np.float32)

    in_maps = []
    for c in range(NCORES):
        h0, h1 = HPC * c, HPC * c + 1
        cols = []
        for h in (h0, h1):
            cols += [w_qkv[h * DK:(h + 1) * DK],            # Q rows
                     w_qkv[D + h * DK:D + (h + 1) * DK]]    # K rows
        # reorder to Q0 K0 Q1 K1 then V0 V1
        cols = [cols[0], cols[1], cols[2], cols[3],
                w_qkv[2 * D + h0 * DK:2 * D + (h0 + 1) * DK],
                w_qkv[2 * D + h1 * DK:2 * D + (h1 + 1) * DK]]
        w = np.ascontiguousarray(
            np.concatenate(cols, 0).T).reshape(KC, P, WC).astype(bf)
        wo = np.ascontiguousarray(
            w_o[:, HPC * DK * c:HPC * DK * (c + 1)].T).astype(bf)
        in_maps.append({
            "xT": xT, "w": w, "woT": wo, "tri": tri, "onr": onr,
        })
    return in_maps


_nc_cache = {}


def get_nc(debug=False, **opts):
    key = (debug, tuple(sorted((k, str(v)) for k, v in opts.items())))
    if key not in _nc_cache:
        _nc_cache[key] = build(debug=debug, **opts)
    return _nc_cache[key]


def run(x, w_qkv, w_o, debug=False, **opts):
    nc = get_nc(debug=debug, **opts)
    in_maps = prep_inputs(x, w_qkv, w_o)
    res = bass_utils.run_bass_kernel_spmd(nc, in_maps, core_ids=list(range(NCORES)))
    return res


def kernel(x, w_qkv, w_o):
    res = run(x, w_qkv, w_o)
    y = res.results[0]["y"].astype(np.float64)
    for c in range(1, NCORES):
        y += res.results[c]["y"]
    return y.astype(np.float32).reshape(B, T, D)


# revision 30
# speedup vs baseline: 1.2449x; 1.0732x over previous
"""Trainium2 Bass kernel for causal multi-head attention (dense transformer block).

Math (reference semantics):
    qkv = x @ w_qkv.T ; split into Q,K,V heads [B,H,T,dk]
    (rotary in the reference rotates Q and K of head h by a constant,
     time-independent orthogonal rotation R_h; since scores = (R_h q)·(R_h k)
     = q·k, the rotation cancels exactly and is skipped here)
    scores = causal_mask(Q @ K.T / sqrt(dk)); attn = softmax(scores)
    out = attn @ V ; y = out @ w_o.T

Sharding: head-parallel over 8 cores (2 heads/core, both batches).  Each core
computes a partial y (its heads' contribution through w_o columns); the host
sums the 8 partials (the "all-reduce").

v3 design (vs the f32r baseline):
  * All matmul inputs bf16 (same 1 cyc/row PE rate as f32r at wide free dims,
    half the DMA + SBUF).  PSUM accumulation stays fp32.
  * Phase 1 is k-outer: 6 concurrent PSUM groups (Q/K for 2 heads in two
    2-bank "S" tiles, V written DIRECTLY in [token, dk] layout into "O"
    tiles) so PE starts as soon as the first weight/x chunk lands and no
    V^T->V transposes are needed.  PSUM drains on ACT/DVE (GpSimd cannot
    touch PSUM), chunk-PAIR DMAs halve HWDGE descriptor-queue pressure.
  * Softmax denominator: bf16 pair-add (DVE 4x mode) + fp32 running sums
    split into two chains (GpSimd + DVE), then two GpSimd cross-partition
    (axis=C) reduces - no [1,512] ones-matmuls on PE (saves ~34us PE).
  * Causal narrowing: diagonal key-tiles only compute the live q-suffix in
    scores/AV/exp; the dead ex prefix is zeroed by a GpSimd memset; the
    128x128 causal triangle is masked by a GpSimd multiply.
  * proj(qb) units are woven between the attention kt-pairs of the next
    unit (qb3 into the next batch's phase 1 + qb0 unit) so the exp-gated
    stretches of attention get PE filler; yt PSUM->SBUF copies alternate
    ACT/DVE; y stores go out as one wide DMA per token tile.
  * softmax close-out chains (reduce -> recip -> broadcast-mm -> normalize)
    are deferred into the following instruction stream so PE (in-order)
    never waits on them.
"""

import contextlib

import numpy as np

import concourse.bacc as bacc
import concourse.bass as bass
import concourse.mybir as mybir
import concourse.tile as tile
from concourse import bass_utils

B, T, D, H, DK = 2, 2048, 2048, 16, 128
NCORES = 8
HPC = H // NCORES  # heads per core
P = 128
NB = 512           # q-block / token-block / e-block width
KC = D // P        # 16 contraction chunks of the model dim
KP = KC // 2       # chunk pairs
QB = T // NB       # 4 q blocks per batch
NT = T // P        # 16 token tiles per batch
WC = 6 * P         # w columns per chunk: Q0 K0 Q1 K1 V0 V1
FP32 = mybir.dt.float32
F32R = mybir.dt.float32r
BF16 = mybir.dt.bfloat16
SCALE = 1.0 / np.sqrt(DK)

DEFAULT_OPTS = dict(
    ex_bufs=3, xt_bufs=2, yt_bufs=3, s_bufs=2, o_bufs=2, y_bufs=2,
    loop_n=1,
    # yt-copy engine cycle per *hosting location* of the proj units
    pat_attn={0: "DA", 1: "DA", 2: "DA", 3: "DA"},
    pat_p1="DA", pat_tail="AD",
    defer_pairs=2,
    chain_pat={0: "PD", 1: "PD", 2: "PD", 3: "DD"},  # per qb (even, odd pair)
    mask_eng="D",
    qkv_q="A", qkv_k="D", qkv_v="AD", pipe=True, chain_bf16=True,
)

_ENG_MAP = {"P": "gpsimd", "A": "scalar", "D": "vector"}


def _width(kt, qb):
    """Live q-suffix width of key tile kt within q-block qb (causal)."""
    j = kt - 4 * qb
    if j <= 0:
        return NB
    return NB - P * j


def build(debug=False, **opts):
    o = dict(DEFAULT_OPTS)
    o.update({k: v for k, v in opts.items() if k in DEFAULT_OPTS})
    nc = bacc.Bacc("TRN2", target_bir_lowering=False, debug=False,
                   num_devices=NCORES)
    # 3D dram layouts allow one DMA per chunk-pair / token tile
    xT = nc.dram_tensor("xT", [KC, P, B * T], BF16, kind="ExternalInput")
    w_d = nc.dram_tensor("w", [KC, P, WC], BF16, kind="ExternalInput")
    woT = nc.dram_tensor("woT", [HPC * DK, D], BF16, kind="ExternalInput")
    tri_d = nc.dram_tensor("tri", [P, P], BF16, kind="ExternalInput")
    onr_d = nc.dram_tensor("onr", [1, P], F32R, kind="ExternalInput")
    y = nc.dram_tensor("y", [B * T, D], BF16, kind="ExternalOutput")
    dbg = {}
    if debug:
        for nm in ("QT", "KT", "outT"):
            dbg[nm] = nc.dram_tensor(f"dbg_{nm}", [HPC, P, T], BF16,
                                     kind="ExternalOutput")
        dbg["V"] = nc.dram_tensor("dbg_V", [NT, P, 2 * P], BF16,
                                  kind="ExternalOutput")

    with tile.TileContext(nc) as tc:
        with (
            tc.tile_pool(name="const", bufs=1) as cpool,
            tc.tile_pool(name="xp", bufs=1) as xpool,
            tc.tile_pool(name="qkv", bufs=1) as qpool,
            tc.tile_pool(name="attn", bufs=1) as apool,
            tc.tile_pool(name="ps", bufs=1, space="PSUM") as pspool,
        ):
            # ---- constants / weights resident in SBUF ----
            w_sb = [cpool.tile([P, 2, WC], BF16, name=f"w_{kp}") for kp in range(KP)]
            wo_sb = [cpool.tile([P, D], BF16, name=f"wo_{h}") for h in range(HPC)]
            tri = cpool.tile([P, P], BF16, name="tri")
            onr = cpool.tile([1, P], F32R, name="onr")

            def wqk(k, m):  # m in 0..3 = Q0 K0 Q1 K1 of chunk k
                return w_sb[k // 2][:, k % 2, m * P:(m + 1) * P]

            def wvv(k):     # V columns (both heads) of chunk k
                return w_sb[k // 2][:, k % 2, 4 * P:6 * P]

            # persistent per-batch state (WAR deps recycle across batches)
            QTp = [qpool.tile([P, T], BF16, name=f"QT{h}") for h in range(HPC)]
            KTp = [qpool.tile([P, T], BF16, name=f"KT{h}") for h in range(HPC)]
            Vp = [qpool.tile([P, 2 * P], BF16, name=f"V{kt}") for kt in range(NT)]
            outTp = [qpool.tile([P, T], BF16, name=f"outT{h}") for h in range(HPC)]

            def ps_tile(tag, shape, name, bufs):
                return pspool.tile(shape, FP32, name=name, tag=tag, bufs=bufs)

            def copy_on(code, dst, src):
                eng = _ENG_MAP[code]
                if eng == "gpsimd":
                    nc.gpsimd.tensor_copy(dst, src)
                elif eng == "scalar":
                    nc.scalar.copy(dst, src)
                else:
                    nc.vector.tensor_copy(dst, src)

            loop_ctx = (tc.For_i(0, o["loop_n"], 1, hint_engines=(
                            mybir.EngineType.PE, mybir.EngineType.Activation,
                            mybir.EngineType.DVE, mybir.EngineType.SP,
                            mybir.EngineType.Pool))
                        if o["loop_n"] > 1 else contextlib.nullcontext())

            if o["loop_n"] > 1:
                # weights/constants loaded once, outside the HW loop
                for kp in range(KP):
                    nc.sync.dma_start(w_sb[kp][:], w_d[2 * kp:2 * kp + 2])
                nc.sync.dma_start(tri[:], tri_d[:, :])
                nc.sync.dma_start(onr[:], onr_d[:, :])
                for h in range(HPC):
                    nc.sync.dma_start(wo_sb[h][:], woT[h * P:(h + 1) * P, :])

            with loop_ctx:
                # ============ phase 1 generator (one token block) ============
                def phase1_nb(b, nb):
                    """QKV projection for token block (b, nb), k-outer.
                    Yields after each chunk-pair (8) + once at the drain."""
                    col0 = b * T + nb * NB
                    xt = [xpool.tile([P, 2, NB], BF16, name=f"x{kp}_{b}_{nb}",
                                     tag=f"x{kp}", bufs=o["xt_bufs"])
                          for kp in range(KP)]
                    first = b == 0 and nb == 0 and o["loop_n"] == 1
                    for kp in range(KP):
                        if first and kp == 0:
                            # column-split: first QK slice lands sooner (row
                            # interleave of the pair DMA is preserved)
                            nc.sync.dma_start(w_sb[0][:, :, 0:2 * P],
                                              w_d[0:2, :, 0:2 * P])
                            nc.sync.dma_start(w_sb[0][:, :, 2 * P:WC],
                                              w_d[0:2, :, 2 * P:WC])
                        elif first:
                            nc.sync.dma_start(w_sb[kp][:], w_d[2 * kp:2 * kp + 2])
                        nc.sync.dma_start(xt[kp][:],
                                          xT[2 * kp:2 * kp + 2, :, col0:col0 + NB])
                        if first and kp == 0:
                            nc.sync.dma_start(tri[:], tri_d[:, :])
                            nc.sync.dma_start(onr[:], onr_d[:, :])
                    if b == 0 and nb == 1 and o["loop_n"] == 1:
                        for h in range(HPC):
                            nc.sync.dma_start(wo_sb[h][:], woT[h * P:(h + 1) * P, :])

                    S0 = ps_tile("S", [P, 2, NB], f"p1s0_{b}_{nb}", o["s_bufs"])
                    S1 = ps_tile("S", [P, 2, NB], f"p1s1_{b}_{nb}", o["s_bufs"])
                    # V token-tile groups need a PSUM bank each (one
                    # accumulation group per bank): two sub-sweeps of 2.
                    V01 = [ps_tile("O", [P, NB], f"p1v{t}_{b}_{nb}", o["o_bufs"])
                           for t in range(2)]
                    for kp in range(KP):
                        for half in range(2):
                            k = 2 * kp + half
                            st, sp = k == 0, k == KC - 1
                            xk = xt[kp][:, half, :]
                            nc.tensor.matmul(S0[:, 0, :], wqk(k, 0), xk,
                                             start=st, stop=sp)
                            nc.tensor.matmul(S0[:, 1, :], wqk(k, 1), xk,
                                             start=st, stop=sp)
                            nc.tensor.matmul(S1[:, 0, :], wqk(k, 2), xk,
                                             start=st, stop=sp)
                            nc.tensor.matmul(S1[:, 1, :], wqk(k, 3), xk,
                                             start=st, stop=sp)
                            for t in range(2):
                                nc.tensor.matmul(V01[t][:, 0:2 * P],
                                                 xt[kp][:, half, t * P:(t + 1) * P],
                                                 wvv(k), start=st, stop=sp)
                        yield
                    # drain QK + first V pair; second V pair sweep follows
                    csl = slice(nb * NB, (nb + 1) * NB)
                    copy_on(o["qkv_q"], QTp[0][:, csl], S0[:, 0, :])
                    copy_on(o["qkv_k"], KTp[0][:, csl], S0[:, 1, :])
                    copy_on(o["qkv_q"], QTp[1][:, csl], S1[:, 0, :])
                    copy_on(o["qkv_k"], KTp[1][:, csl], S1[:, 1, :])
                    for t in range(2):
                        copy_on(o["qkv_v"][t % len(o["qkv_v"])],
                                Vp[nb * 4 + t][:], V01[t][:, 0:2 * P])
                    yield
                    V23 = [ps_tile("O", [P, NB], f"p1v{2 + t}_{b}_{nb}",
                                   o["o_bufs"]) for t in range(2)]
                    for kp in range(KP):
                        for half in range(2):
                            k = 2 * kp + half
                            st, sp = k == 0, k == KC - 1
                            for t in range(2):
                                nc.tensor.matmul(V23[t][:, 0:2 * P],
                                                 xt[kp][:, half,
                                                        (2 + t) * P:(3 + t) * P],
                                                 wvv(k), start=st, stop=sp)
                        if kp % 2 == 1:
                            yield
                    for t in range(2):
                        copy_on(o["qkv_v"][t % len(o["qkv_v"])],
                                Vp[nb * 4 + 2 + t][:], V23[t][:, 0:2 * P])
                    yield

                # ============ attention generator (one head) ============
                def attention_gen(b, h, qb):
                    """Yields once per kt-pair.  Returns the deferred
                    close-out thunk (bcmm + normalize)."""
                    nkt = 4 * qb + 4
                    qsl0 = qb * NB
                    ps_o = ps_tile("O", [P, NB], f"pso_{b}_{h}_{qb}", o["o_bufs"])
                    # two running-sum chains: even pairs / odd pairs
                    cdt = BF16 if o["chain_bf16"] else FP32
                    exs = [apool.tile([P, NB], cdt, name=f"exs{i}_{b}_{h}_{qb}",
                                      tag=f"exsum{i}", bufs=2) for i in range(2)]
                    npair = nkt // 2

                    def emit_scores_exp(p):
                        a, c = 2 * p, 2 * p + 1
                        oa, oc = NB - _width(a, qb), NB - _width(c, qb)
                        ps_s = ps_tile("S", [P, 2, NB], f"pss_{b}_{h}_{qb}_{p}",
                                       o["s_bufs"])
                        nc.tensor.matmul(ps_s[:, 0, oa:NB],
                                         KTp[h][:, a * P:(a + 1) * P],
                                         QTp[h][:, qsl0 + oa:qsl0 + NB],
                                         start=True, stop=True)
                        nc.tensor.matmul(ps_s[:, 1, oc:NB],
                                         KTp[h][:, c * P:(c + 1) * P],
                                         QTp[h][:, qsl0 + oc:qsl0 + NB],
                                         start=True, stop=True)
                        ex = apool.tile([P, 2, NB], BF16,
                                        name=f"ex_{b}_{h}_{qb}_{p}",
                                        tag="ex", bufs=o["ex_bufs"])
                        if oa == oc:
                            nc.scalar.activation(ex[:, :, oa:NB],
                                                 ps_s[:, :, oa:NB],
                                                 mybir.ActivationFunctionType.Exp,
                                                 scale=SCALE)
                        else:
                            nc.scalar.activation(ex[:, 0, oa:NB],
                                                 ps_s[:, 0, oa:NB],
                                                 mybir.ActivationFunctionType.Exp,
                                                 scale=SCALE)
                            nc.scalar.activation(ex[:, 1, oc:NB],
                                                 ps_s[:, 1, oc:NB],
                                                 mybir.ActivationFunctionType.Exp,
                                                 scale=SCALE)
                        return (p, ex, oa, oc)

                    def emit_post(st):
                        p, ex, oa, oc = st
                        a, c = 2 * p, 2 * p + 1
                        # zero dead prefixes of narrowed (diagonal) tiles
                        if oa > 0:
                            nc.gpsimd.memset(ex[:, 0, 0:oa], 0.0)
                        if oc > 0:
                            nc.gpsimd.memset(ex[:, 1, 0:oc], 0.0)
                        # triangle masks on diagonal tiles
                        for half, kt, off in ((0, a, oa), (1, c, oc)):
                            if kt >= 4 * qb:
                                sl = ex[:, half, off:off + P]
                                if o["mask_eng"] == "P":
                                    nc.gpsimd.tensor_mul(sl, sl, tri[:])
                                else:
                                    nc.vector.tensor_mul(sl, sl, tri[:])
                        # denominator partial: exs[p%2] += ex.lo + ex.hi
                        tpr = apool.tile([P, NB], BF16,
                                         name=f"tp_{b}_{h}_{qb}_{p}",
                                         tag="tpr", bufs=2)
                        nc.vector.tensor_add(tpr[:], ex[:, 0, :], ex[:, 1, :])
                        cp = o["chain_pat"][qb] if isinstance(o["chain_pat"], dict) else o["chain_pat"]
                        eng = getattr(nc, _ENG_MAP[cp[p % 2]])
                        if p < 2:
                            eng.tensor_copy(exs[p % 2][:], tpr[:])
                        else:
                            eng.tensor_add(exs[p % 2][:], exs[p % 2][:], tpr[:])
                        # AV accumulation
                        nc.tensor.matmul(ps_o[:, oa:NB],
                                         Vp[a][:, h * P:(h + 1) * P],
                                         ex[:, 0, oa:NB],
                                         start=(p == 0), stop=False,
                                         skip_group_check=True)
                        nc.tensor.matmul(ps_o[:, oc:NB],
                                         Vp[c][:, h * P:(h + 1) * P],
                                         ex[:, 1, oc:NB],
                                         start=False, stop=(p == npair - 1),
                                         skip_group_check=True)

                    # software pipeline: scores/exp of p+1 before AV of p
                    if o["pipe"]:
                        st = emit_scores_exp(0)
                        for p in range(npair):
                            nxt = (emit_scores_exp(p + 1)
                                   if p + 1 < npair else None)
                            emit_post(st)
                            st = nxt
                            yield
                    else:
                        for p in range(npair):
                            emit_post(emit_scores_exp(p))
                            yield
                    # denominator: merge chains, cross-partition reduce, recip
                    rec = apool.tile([1, NB], F32R, name=f"rec_{b}_{h}_{qb}",
                                     tag="rec", bufs=2)
                    dn = apool.tile([1, NB], FP32, name=f"dn_{b}_{h}_{qb}",
                                    tag="dn", bufs=2)
                    if o["chain_bf16"]:
                        mrg = apool.tile([P, NB], BF16, name=f"mg_{b}_{h}_{qb}",
                                         tag="mrg", bufs=2)
                        nc.vector.tensor_add(mrg[:], exs[0][:], exs[1][:])
                        nc.gpsimd.tensor_reduce(dn[:1, :], mrg[:],
                                                axis=mybir.AxisListType.C,
                                                op=mybir.AluOpType.add)
                    else:
                        dn1 = apool.tile([1, NB], FP32, name=f"dn1_{b}_{h}_{qb}",
                                         tag="dn1", bufs=2)
                        nc.gpsimd.tensor_reduce(dn[:1, :], exs[0][:],
                                                axis=mybir.AxisListType.C,
                                                op=mybir.AluOpType.add)
                        nc.gpsimd.tensor_reduce(dn1[:1, :], exs[1][:],
                                                axis=mybir.AxisListType.C,
                                                op=mybir.AluOpType.add)
                        nc.vector.tensor_add(dn[:1, :], dn[:1, :], dn1[:1, :])
                    with nc.allow_low_precision(reason="f32r recip: tf32 ok"):
                        nc.vector.reciprocal(rec[:1, :], dn[:1, :])

                    def close():
                        ps_bc = ps_tile("Y", [P, NB], f"psbc_{b}_{h}_{qb}",
                                        o["y_bufs"])
                        nc.tensor.matmul(ps_bc[:], onr[:1, :], rec[:1, :],
                                         start=True, stop=True)
                        bc = apool.tile([P, NB], FP32, name=f"bc_{b}_{h}_{qb}",
                                        tag="bc", bufs=2)
                        nc.vector.tensor_copy(bc[:], ps_bc[:])
                        nc.vector.tensor_mul(outTp[h][:, qsl0:qsl0 + NB],
                                             ps_o[:], bc[:])
                    return close

                # ============ proj units ============
                yts = {}

                def proj_unit(b, tt, eb, eng_code, narrow_dma=False):
                    ps_y = ps_tile("Y", [P, NB], f"psy_{b}_{tt}_{eb}", o["y_bufs"])
                    nc.tensor.matmul(ps_y[:], outTp[0][:, tt * P:(tt + 1) * P],
                                     wo_sb[0][:, eb * NB:(eb + 1) * NB],
                                     start=True, stop=False)
                    nc.tensor.matmul(ps_y[:], outTp[1][:, tt * P:(tt + 1) * P],
                                     wo_sb[1][:, eb * NB:(eb + 1) * NB],
                                     start=False, stop=True)
                    if eb == 0:
                        yts[(b, tt)] = apool.tile([P, QB, NB], BF16,
                                                  name=f"yt_{b}_{tt}",
                                                  tag="yt", bufs=o["yt_bufs"])
                    yt = yts[(b, tt)]
                    copy_on(eng_code, yt[:, eb, :], ps_y[:])
                    if narrow_dma:
                        # tail: stream each e-block out as soon as copied
                        nc.sync.dma_start(
                            y[b * T + tt * P:b * T + (tt + 1) * P,
                              eb * NB:(eb + 1) * NB], yt[:, eb, :])
                    elif eb == QB - 1:
                        nc.sync.dma_start(
                            y[b * T + tt * P:b * T + (tt + 1) * P, :], yt[:])

                def proj_thunks(b, pqb, pat, narrow_dma=False):
                    th = []
                    i = 0
                    for tt in range(4 * pqb, 4 * pqb + 4):
                        for eb in range(QB):
                            code = pat[i % len(pat)]
                            th.append(lambda b=b, tt=tt, eb=eb, code=code:
                                      proj_unit(b, tt, eb, code, narrow_dma))
                            i += 1
                    return th

                # ============ weaving driver ============
                def weave(gen, fillers, carry, defer=None):
                    """Run gen; after each yield emit carry thunks (once,
                    after o['defer_pairs'] yields) and a fair share of
                    fillers (popped from the shared list)."""
                    n = 0
                    held = 0
                    try:
                        while True:
                            next(gen)
                            n += 1
                            if n >= o["defer_pairs"] and carry:
                                for fn in carry:
                                    fn()
                                carry = []
                            if defer:
                                held += defer.pop(0)
                            if not carry:
                                while held > 0 and fillers:
                                    fillers.pop(0)()
                                    held -= 1
                    except StopIteration as si:
                        for fn in carry:
                            fn()
                        return si.value

                def share(nfill, nsteps):
                    base, rem = divmod(nfill, nsteps)
                    return [base + (1 if i < rem else 0) for i in range(nsteps)]

                def attention_unit(b, qb, fillers, carry, reserve=3):
                    npair = 2 * qb + 2
                    # hold a few fillers back to cover the close-out chain
                    # latency after the last AV pair
                    nres = min(reserve, len(fillers))
                    sh = share(len(fillers) - nres, 2 * npair)
                    close0 = weave(attention_gen(b, 0, qb), fillers, carry,
                                   defer=sh[:npair])
                    close1 = weave(attention_gen(b, 1, qb), fillers, [close0],
                                   defer=sh[npair:])
                    for fn in fillers:  # reserved + leftovers
                        fn()
                    del fillers[:]
                    return [close1]

                # ============ main schedule ============
                carry = []
                for b in range(B):
                    # ---- phase 1 (+ second half of prev batch qb3 proj) ----
                    if b > 0:
                        ph1_fill = proj_thunks(b - 1, 3, o["pat_p1"])[8:]
                        qb0_fill = proj_thunks(b - 1, 3, o["pat_attn"][3])[:8]
                    else:
                        ph1_fill, qb0_fill = [], []
                    for nb in range(QB):
                        weave(phase1_nb(b, nb), ph1_fill, carry,
                              defer=share(2, KP + 1) if ph1_fill else None)
                        carry = []
                    for fn in ph1_fill:
                        fn()
                    # ---- attention + proj weave ----
                    carry = attention_unit(b, 0, qb0_fill, carry)
                    carry = attention_unit(
                        b, 1, proj_thunks(b, 0, o["pat_attn"][0]), carry)
                    carry = attention_unit(
                        b, 2, proj_thunks(b, 1, o["pat_attn"][1]), carry)
                    carry = attention_unit(
                        b, 3, proj_thunks(b, 2, o["pat_attn"][2]), carry)
                # ---- tail: close-out then final batch qb3 proj ----
                for c in carry:
                    c()
                carry = []
                for fn in proj_thunks(B - 1, 3, o["pat_tail"],
                                      narrow_dma=True):
                    fn()
                if debug:
                    for h in range(HPC):
                        nc.sync.dma_start(dbg["QT"][h], QTp[h][:])
                        nc.sync.dma_start(dbg["KT"][h], KTp[h][:])
                        nc.sync.dma_start(dbg["outT"][h], outTp[h][:])
                    for kt in range(NT):
                        nc.sync.dma_start(dbg["V"][kt], Vp[kt][:])

    nc.compile()
    return nc


def prep_inputs(x, w_qkv, w_o):
    """Host-side shard prep. Returns per-core input maps (bf16)."""
    bf = mybir.dt.np(BF16)
    x = np.asarray(x, dtype=np.float32).reshape(B * T, D)
    xT = np.ascontiguousarray(x.T).reshape(KC, P, B * T).astype(bf)
    w_qkv = np.asarray(w_qkv, dtype=np.float32)
    w_o = np.asarray(w_o, dtype=np.float32)

    tri = np.zeros((P, P), dtype=np.float32)
    kp = np.arange(P)[:, None]
    qu = np.arange(P)[None, :]
    tri[kp <= qu] = 1.0
    tri = tri.astype(bf)
    onr = np.ones((1, P), dtype=np.float32)

    in_maps = []
    for c in range(NCORES):
        h0, h1 = HPC * c, HPC * c + 1
        cols = []
        for h in (h0, h1):
            cols += [w_qkv[h * DK:(h + 1) * DK],            # Q rows
                     w_qkv[D + h * DK:D + (h + 1) * DK]]    # K rows
        # reorder to Q0 K0 Q1 K1 then V0 V1
        cols = [cols[0], cols[1], cols[2], cols[3],
                w_qkv[2 * D + h0 * DK:2 * D + (h0 + 1) * DK],
                w_qkv[2 * D + h1 * DK:2 * D + (h1 + 1) * DK]]
        w = np.ascontiguousarray(
            np.concatenate(cols, 0).T).reshape(KC, P, WC).astype(bf)
        wo = np.ascontiguousarray(
            w_o[:, HPC * DK * c:HPC * DK * (c + 1)].T).astype(bf)
        in_maps.append({
            "xT": xT, "w": w, "woT": wo, "tri": tri, "onr": onr,
        })
    return in_maps


_nc_cache = {}


def get_nc(debug=False, **opts):
    key = (debug, tuple(sorted((k, str(v)) for k, v in opts.items())))
    if key not in _nc_cache:
        _nc_cache[key] = build(debug=debug, **opts)
    return _nc_cache[key]


def run(x, w_qkv, w_o, debug=False, **opts):
    nc = get_nc(debug=debug, **opts)
    in_maps = prep_inputs(x, w_qkv, w_o)
    res = bass_utils.run_bass_kernel_spmd(nc, in_maps, core_ids=list(range(NCORES)))
    return res


def kernel(x, w_qkv, w_o):
    res = run(x, w_qkv, w_o)
    y = res.results[0]["y"].astype(np.float64)
    for c in range(1, NCORES):
        y += res.results[c]["y"]
    return y.astype(np.float32).reshape(B, T, D)


# revision 39
# speedup vs baseline: 1.2755x; 1.0245x over previous
"""Trainium2 Bass kernel for causal multi-head attention (dense transformer block).

Math (reference semantics):
    qkv = x @ w_qkv.T ; split into Q,K,V heads [B,H,T,dk]
    (rotary in the reference rotates Q and K of head h by a constant,
     time-independent orthogonal rotation R_h; since scores = (R_h q)·(R_h k)
     = q·k, the rotation cancels exactly and is skipped here)
    scores = causal_mask(Q @ K.T / sqrt(dk)); attn = softmax(scores)
    out = attn @ V ; y = out @ w_o.T

Sharding: head-parallel over 8 cores (2 heads/core, both batches).  Each core
computes a partial y (its heads' contribution through w_o columns); the host
sums the 8 partials (the "all-reduce").

v3 design (vs the f32r baseline):
  * All matmul inputs bf16 (same 1 cyc/row PE rate as f32r at wide free dims,
    half the DMA + SBUF).  PSUM accumulation stays fp32.
  * Phase 1 is k-outer: 6 concurrent PSUM groups (Q/K for 2 heads in two
    2-bank "S" tiles, V written DIRECTLY in [token, dk] layout into "O"
    tiles) so PE starts as soon as the first weight/x chunk lands and no
    V^T->V transposes are needed.  PSUM drains on ACT/DVE (GpSimd cannot
    touch PSUM), chunk-PAIR DMAs halve HWDGE descriptor-queue pressure.
  * Softmax denominator: bf16 pair-add (DVE 4x mode) + fp32 running sums
    split into two chains (GpSimd + DVE), then two GpSimd cross-partition
    (axis=C) reduces - no [1,512] ones-matmuls on PE (saves ~34us PE).
  * Causal narrowing: diagonal key-tiles only compute the live q-suffix in
    scores/AV/exp; the dead ex prefix is zeroed by a GpSimd memset; the
    128x128 causal triangle is masked by a GpSimd multiply.
  * proj(qb) units are woven between the attention kt-pairs of the next
    unit (qb3 into the next batch's phase 1 + qb0 unit) so the exp-gated
    stretches of attention get PE filler; yt PSUM->SBUF copies alternate
    ACT/DVE; y stores go out as one wide DMA per token tile.
  * softmax close-out chains (reduce -> recip -> broadcast-mm -> normalize)
    are deferred into the following instruction stream so PE (in-order)
    never waits on them.
"""

import contextlib

import numpy as np

import concourse.bacc as bacc
import concourse.bass as bass
import concourse.mybir as mybir
import concourse.tile as tile
from concourse import bass_utils

B, T, D, H, DK = 2, 2048, 2048, 16, 128
NCORES = 8
HPC = H // NCORES  # heads per core
P = 128
NB = 512           # q-block / token-block / e-block width
KC = D // P        # 16 contraction chunks of the model dim
KP = KC // 2       # chunk pairs
QB = T // NB       # 4 q blocks per batch
NT = T // P        # 16 token tiles per batch
WC = 6 * P         # w columns per chunk: Q0 K0 Q1 K1 V0 V1
FP32 = mybir.dt.float32
F32R = mybir.dt.float32r
BF16 = mybir.dt.bfloat16
SCALE = 1.0 / np.sqrt(DK)

DEFAULT_OPTS = dict(
    ex_bufs=4, xt_bufs=2, yt_bufs=3, s_bufs=2, o_bufs=2, y_bufs=2,
    loop_n=1,
    # yt-copy engine cycle per *hosting location* of the proj units
    pat_attn={0: "DA", 1: "DA", 2: "DA", 3: "DA"},
    pat_p1="DA", pat_tail="AD",
    defer_pairs=2,
    chain_pat={0: "PD", 1: "PD", 2: "PD", 3: "DD"},  # per qb (even, odd pair)
    mask_eng="D",
    qkv_q="A", qkv_k="D", qkv_v="AD", pipe=True, chain_bf16=True,
)

_ENG_MAP = {"P": "gpsimd", "A": "scalar", "D": "vector"}


def _width(kt, qb):
    """Live q-suffix width of key tile kt within q-block qb (causal)."""
    j = kt - 4 * qb
    if j <= 0:
        return NB
    return NB - P * j


def build(debug=False, **opts):
    o = dict(DEFAULT_OPTS)
    o.update({k: v for k, v in opts.items() if k in DEFAULT_OPTS})
    nc = bacc.Bacc("TRN2", target_bir_lowering=False, debug=False,
                   num_devices=NCORES)
    # 3D dram layouts allow one DMA per chunk-pair / token tile
    xT = nc.dram_tensor("xT", [KC, P, B * T], BF16, kind="ExternalInput")
    w_d = nc.dram_tensor("w", [KC, P, WC], BF16, kind="ExternalInput")
    woT = nc.dram_tensor("woT", [HPC * DK, D], BF16, kind="ExternalInput")
    tri_d = nc.dram_tensor("tri", [P, P], BF16, kind="ExternalInput")
    onr_d = nc.dram_tensor("onr", [1, P], F32R, kind="ExternalInput")
    y = nc.dram_tensor("y", [B * T, D], BF16, kind="ExternalOutput")
    dbg = {}
    if debug:
        for nm in ("QT", "KT", "outT"):
            dbg[nm] = nc.dram_tensor(f"dbg_{nm}", [HPC, P, T], BF16,
                                     kind="ExternalOutput")
        dbg["V"] = nc.dram_tensor("dbg_V", [NT, P, 2 * P], BF16,
                                  kind="ExternalOutput")

    with tile.TileContext(nc) as tc:
        with (
            tc.tile_pool(name="const", bufs=1) as cpool,
            tc.tile_pool(name="xp", bufs=1) as xpool,
            tc.tile_pool(name="qkv", bufs=1) as qpool,
            tc.tile_pool(name="attn", bufs=1) as apool,
            tc.tile_pool(name="ps", bufs=1, space="PSUM") as pspool,
        ):
            # ---- constants / weights resident in SBUF ----
            w_sb = [cpool.tile([P, 2, WC], BF16, name=f"w_{kp}") for kp in range(KP)]
            wo_sb = [cpool.tile([P, D], BF16, name=f"wo_{h}") for h in range(HPC)]
            tri = cpool.tile([P, P], BF16, name="tri")
            onr = cpool.tile([1, P], F32R, name="onr")

            def wqk(k, m):  # m in 0..3 = Q0 K0 Q1 K1 of chunk k
                return w_sb[k // 2][:, k % 2, m * P:(m + 1) * P]

            def wvv(k):     # V columns (both heads) of chunk k
                return w_sb[k // 2][:, k % 2, 4 * P:6 * P]

            # persistent per-batch state (WAR deps recycle across batches)
            QTp = [qpool.tile([P, T], BF16, name=f"QT{h}") for h in range(HPC)]
            KTp = [qpool.tile([P, T], BF16, name=f"KT{h}") for h in range(HPC)]
            Vp = [qpool.tile([P, 2 * P], BF16, name=f"V{kt}") for kt in range(NT)]
            outTp = [qpool.tile([P, T], BF16, name=f"outT{h}") for h in range(HPC)]

            def ps_tile(tag, shape, name, bufs):
                return pspool.tile(shape, FP32, name=name, tag=tag, bufs=bufs)

            def copy_on(code, dst, src):
                eng = _ENG_MAP[code]
                if eng == "gpsimd":
                    nc.gpsimd.tensor_copy(dst, src)
                elif eng == "scalar":
                    nc.scalar.copy(dst, src)
                else:
                    nc.vector.tensor_copy(dst, src)

            loop_ctx = (tc.For_i(0, o["loop_n"], 1, hint_engines=(
                            mybir.EngineType.PE, mybir.EngineType.Activation,
                            mybir.EngineType.DVE, mybir.EngineType.SP,
                            mybir.EngineType.Pool))
                        if o["loop_n"] > 1 else contextlib.nullcontext())

            if o["loop_n"] > 1:
                # weights/constants loaded once, outside the HW loop
                for kp in range(KP):
                    nc.sync.dma_start(w_sb[kp][:], w_d[2 * kp:2 * kp + 2])
                nc.sync.dma_start(tri[:], tri_d[:, :])
                nc.sync.dma_start(onr[:], onr_d[:, :])
                for h in range(HPC):
                    nc.sync.dma_start(wo_sb[h][:], woT[h * P:(h + 1) * P, :])

            with loop_ctx:
                # ============ phase 1 generator (one token block) ============
                def phase1_nb(b, nb):
                    """QKV projection for token block (b, nb), k-outer.
                    Yields after each chunk-pair (8) + once at the drain."""
                    col0 = b * T + nb * NB
                    xt = [xpool.tile([P, 2, NB], BF16, name=f"x{kp}_{b}_{nb}",
                                     tag=f"x{kp}", bufs=o["xt_bufs"])
                          for kp in range(KP)]
                    first = b == 0 and nb == 0 and o["loop_n"] == 1
                    for kp in range(KP):
                        if first and kp == 0:
                            # column-split: first QK slice lands sooner (row
                            # interleave of the pair DMA is preserved)
                            nc.sync.dma_start(w_sb[0][:, :, 0:2 * P],
                                              w_d[0:2, :, 0:2 * P])
                            nc.sync.dma_start(w_sb[0][:, :, 2 * P:WC],
                                              w_d[0:2, :, 2 * P:WC])
                        elif first:
                            nc.sync.dma_start(w_sb[kp][:], w_d[2 * kp:2 * kp + 2])
                        nc.sync.dma_start(xt[kp][:],
                                          xT[2 * kp:2 * kp + 2, :, col0:col0 + NB])
                        if first and kp == 0:
                            nc.sync.dma_start(tri[:], tri_d[:, :])
                            nc.sync.dma_start(onr[:], onr_d[:, :])
                    if b == 0 and nb == 1 and o["loop_n"] == 1:
                        for h in range(HPC):
                            nc.sync.dma_start(wo_sb[h][:], woT[h * P:(h + 1) * P, :])

                    S0 = ps_tile("S", [P, 2, NB], f"p1s0_{b}_{nb}", o["s_bufs"])
                    S1 = ps_tile("S", [P, 2, NB], f"p1s1_{b}_{nb}", o["s_bufs"])
                    # V token-tile groups need a PSUM bank each (one
                    # accumulation group per bank): two sub-sweeps of 2.
                    V01 = [ps_tile("O", [P, NB], f"p1v{t}_{b}_{nb}", o["o_bufs"])
                           for t in range(2)]
                    for kp in range(KP):
                        for half in range(2):
                            k = 2 * kp + half
                            st, sp = k == 0, k == KC - 1
                            xk = xt[kp][:, half, :]
                            nc.tensor.matmul(S0[:, 0, :], wqk(k, 0), xk,
                                             start=st, stop=sp)
                            nc.tensor.matmul(S0[:, 1, :], wqk(k, 1), xk,
                                             start=st, stop=sp)
                            nc.tensor.matmul(S1[:, 0, :], wqk(k, 2), xk,
                                             start=st, stop=sp)
                            nc.tensor.matmul(S1[:, 1, :], wqk(k, 3), xk,
                                             start=st, stop=sp)
                            for t in range(2):
                                nc.tensor.matmul(V01[t][:, 0:2 * P],
                                                 xt[kp][:, half, t * P:(t + 1) * P],
                                                 wvv(k), start=st, stop=sp)
                        yield
                    # drain QK + first V pair; second V pair sweep follows
                    csl = slice(nb * NB, (nb + 1) * NB)
                    copy_on(o["qkv_q"], QTp[0][:, csl], S0[:, 0, :])
                    copy_on(o["qkv_k"], KTp[0][:, csl], S0[:, 1, :])
                    copy_on(o["qkv_q"], QTp[1][:, csl], S1[:, 0, :])
                    copy_on(o["qkv_k"], KTp[1][:, csl], S1[:, 1, :])
                    for t in range(2):
                        copy_on(o["qkv_v"][t % len(o["qkv_v"])],
                                Vp[nb * 4 + t][:], V01[t][:, 0:2 * P])
                    yield
                    # V2 then V3 sequentially: each holds only ONE O slot,
                    # so attention(qb0) can interleave using the other slot
                    for t in range(2, 4):
                        Vt = ps_tile("O", [P, NB], f"p1v{t}_{b}_{nb}",
                                     o["o_bufs"])
                        for kp in range(KP):
                            for half in range(2):
                                k = 2 * kp + half
                                nc.tensor.matmul(Vt[:, 0:2 * P],
                                                 xt[kp][:, half,
                                                        t * P:(t + 1) * P],
                                                 wvv(k), start=(k == 0),
                                                 stop=(k == KC - 1))
                            if kp % 2 == 1:
                                yield
                        copy_on(o["qkv_v"][t % len(o["qkv_v"])],
                                Vp[nb * 4 + t][:], Vt[:, 0:2 * P])
                        yield

                # ============ attention generator (one head) ============
                def attention_gen(b, h, qb):
                    """Yields once per kt-pair.  Returns the deferred
                    close-out thunk (bcmm + normalize)."""
                    nkt = 4 * qb + 4
                    qsl0 = qb * NB
                    ps_o = ps_tile("O", [P, NB], f"pso_{b}_{h}_{qb}", o["o_bufs"])
                    # two running-sum chains: even pairs / odd pairs
                    cdt = BF16 if o["chain_bf16"] else FP32
                    exs = [apool.tile([P, NB], cdt, name=f"exs{i}_{b}_{h}_{qb}",
                                      tag=f"exsum{i}", bufs=2) for i in range(2)]
                    npair = nkt // 2

                    def emit_scores_exp(p):
                        a, c = 2 * p, 2 * p + 1
                        oa, oc = NB - _width(a, qb), NB - _width(c, qb)
                        ps_s = ps_tile("S", [P, 2, NB], f"pss_{b}_{h}_{qb}_{p}",
                                       o["s_bufs"])
                        nc.tensor.matmul(ps_s[:, 0, oa:NB],
                                         KTp[h][:, a * P:(a + 1) * P],
                                         QTp[h][:, qsl0 + oa:qsl0 + NB],
                                         start=True, stop=True)
                        nc.tensor.matmul(ps_s[:, 1, oc:NB],
                                         KTp[h][:, c * P:(c + 1) * P],
                                         QTp[h][:, qsl0 + oc:qsl0 + NB],
                                         start=True, stop=True)
                        ex = apool.tile([P, 2, NB], BF16,
                                        name=f"ex_{b}_{h}_{qb}_{p}",
                                        tag="ex", bufs=o["ex_bufs"])
                        if oa == oc:
                            nc.scalar.activation(ex[:, :, oa:NB],
                                                 ps_s[:, :, oa:NB],
                                                 mybir.ActivationFunctionType.Exp,
                                                 scale=SCALE)
                        else:
                            nc.scalar.activation(ex[:, 0, oa:NB],
                                                 ps_s[:, 0, oa:NB],
                                                 mybir.ActivationFunctionType.Exp,
                                                 scale=SCALE)
                            nc.scalar.activation(ex[:, 1, oc:NB],
                                                 ps_s[:, 1, oc:NB],
                                                 mybir.ActivationFunctionType.Exp,
                                                 scale=SCALE)
                        return (p, ex, oa, oc)

                    def emit_post(st):
                        p, ex, oa, oc = st
                        a, c = 2 * p, 2 * p + 1
                        # zero dead prefixes of narrowed (diagonal) tiles
                        if oa > 0:
                            nc.gpsimd.memset(ex[:, 0, 0:oa], 0.0)
                        if oc > 0:
                            nc.gpsimd.memset(ex[:, 1, 0:oc], 0.0)
                        # triangle masks on diagonal tiles
                        for half, kt, off in ((0, a, oa), (1, c, oc)):
                            if kt >= 4 * qb:
                                sl = ex[:, half, off:off + P]
                                if o["mask_eng"] == "P":
                                    nc.gpsimd.tensor_mul(sl, sl, tri[:])
                                else:
                                    nc.vector.tensor_mul(sl, sl, tri[:])
                        # denominator partial: exs[p%2] += ex.lo + ex.hi
                        tpr = apool.tile([P, NB], BF16,
                                         name=f"tp_{b}_{h}_{qb}_{p}",
                                         tag="tpr", bufs=2)
                        nc.vector.tensor_add(tpr[:], ex[:, 0, :], ex[:, 1, :])
                        cp = o["chain_pat"][qb] if isinstance(o["chain_pat"], dict) else o["chain_pat"]
                        eng = getattr(nc, _ENG_MAP[cp[p % 2]])
                        if p < 2:
                            eng.tensor_copy(exs[p % 2][:], tpr[:])
                        else:
                            eng.tensor_add(exs[p % 2][:], exs[p % 2][:], tpr[:])
                        # AV accumulation
                        nc.tensor.matmul(ps_o[:, oa:NB],
                                         Vp[a][:, h * P:(h + 1) * P],
                                         ex[:, 0, oa:NB],
                                         start=(p == 0), stop=False,
                                         skip_group_check=True)
                        nc.tensor.matmul(ps_o[:, oc:NB],
                                         Vp[c][:, h * P:(h + 1) * P],
                                         ex[:, 1, oc:NB],
                                         start=False, stop=(p == npair - 1),
                                         skip_group_check=True)

                    # software pipeline: scores/exp of p+1 before AV of p
                    if o["pipe"]:
                        st = emit_scores_exp(0)
                        for p in range(npair):
                            nxt = (emit_scores_exp(p + 1)
                                   if p + 1 < npair else None)
                            emit_post(st)
                            st = nxt
                            yield
                    else:
                        for p in range(npair):
                            emit_post(emit_scores_exp(p))
                            yield
                    # denominator: merge chains, cross-partition reduce, recip
                    rec = apool.tile([1, NB], F32R, name=f"rec_{b}_{h}_{qb}",
                                     tag="rec", bufs=2)
                    dn = apool.tile([1, NB], FP32, name=f"dn_{b}_{h}_{qb}",
                                    tag="dn", bufs=2)
                    if o["chain_bf16"]:
                        mrg = apool.tile([P, NB], BF16, name=f"mg_{b}_{h}_{qb}",
                                         tag="mrg", bufs=2)
                        nc.vector.tensor_add(mrg[:], exs[0][:], exs[1][:])
                        nc.gpsimd.tensor_reduce(dn[:1, :], mrg[:],
                                                axis=mybir.AxisListType.C,
                                                op=mybir.AluOpType.add)
                    else:
                        dn1 = apool.tile([1, NB], FP32, name=f"dn1_{b}_{h}_{qb}",
                                         tag="dn1", bufs=2)
                        nc.gpsimd.tensor_reduce(dn[:1, :], exs[0][:],
                                                axis=mybir.AxisListType.C,
                                                op=mybir.AluOpType.add)
                        nc.gpsimd.tensor_reduce(dn1[:1, :], exs[1][:],
                                                axis=mybir.AxisListType.C,
                                                op=mybir.AluOpType.add)
                        nc.vector.tensor_add(dn[:1, :], dn[:1, :], dn1[:1, :])
                    with nc.allow_low_precision(reason="f32r recip: tf32 ok"):
                        nc.vector.reciprocal(rec[:1, :], dn[:1, :])

                    def close():
                        ps_bc = ps_tile("Y", [P, NB], f"psbc_{b}_{h}_{qb}",
                                        o["y_bufs"])
                        nc.tensor.matmul(ps_bc[:], onr[:1, :], rec[:1, :],
                                         start=True, stop=True)
                        bc = apool.tile([P, NB], FP32, name=f"bc_{b}_{h}_{qb}",
                                        tag="bc", bufs=2)
                        nc.vector.tensor_copy(bc[:], ps_bc[:])
                        nc.vector.tensor_mul(outTp[h][:, qsl0:qsl0 + NB],
                                             ps_o[:], bc[:])
                    return close

                # ============ proj units ============
                yts = {}

                def proj_unit(b, tt, eb, eng_code, narrow_dma=False,
                              tag="Y"):
                    ps_y = ps_tile(tag, [P, NB], f"psy_{b}_{tt}_{eb}",
                                   o["y_bufs"])
                    nc.tensor.matmul(ps_y[:], outTp[0][:, tt * P:(tt + 1) * P],
                                     wo_sb[0][:, eb * NB:(eb + 1) * NB],
                                     start=True, stop=False)
                    nc.tensor.matmul(ps_y[:], outTp[1][:, tt * P:(tt + 1) * P],
                                     wo_sb[1][:, eb * NB:(eb + 1) * NB],
                                     start=False, stop=True)
                    if eb == 0:
                        yts[(b, tt)] = apool.tile([P, QB, NB], BF16,
                                                  name=f"yt_{b}_{tt}",
                                                  tag="yt", bufs=o["yt_bufs"])
                    yt = yts[(b, tt)]
                    copy_on(eng_code, yt[:, eb, :], ps_y[:])
                    if narrow_dma:
                        # tail: stream each e-block out as soon as copied
                        nc.sync.dma_start(
                            y[b * T + tt * P:b * T + (tt + 1) * P,
                              eb * NB:(eb + 1) * NB], yt[:, eb, :])
                    elif eb == QB - 1:
                        nc.sync.dma_start(
                            y[b * T + tt * P:b * T + (tt + 1) * P, :], yt[:])

                def proj_thunks(b, pqb, pat, narrow_dma=False, tags="Y"):
                    th = []
                    i = 0
                    for tt in range(4 * pqb, 4 * pqb + 4):
                        for eb in range(QB):
                            code = pat[i % len(pat)]
                            tag = tags[i % len(tags)]
                            th.append(lambda b=b, tt=tt, eb=eb, code=code,
                                      tag=tag:
                                      proj_unit(b, tt, eb, code, narrow_dma,
                                                tag))
                            i += 1
                    return th

                # ============ weaving driver ============
                def weave(gen, fillers, carry, defer=None):
                    """Run gen; after each yield emit carry thunks (once,
                    after o['defer_pairs'] yields) and a fair share of
                    fillers (popped from the shared list)."""
                    n = 0
                    held = 0
                    try:
                        while True:
                            next(gen)
                            n += 1
                            if n >= o["defer_pairs"] and carry:
                                for fn in carry:
                                    fn()
                                carry = []
                            if defer:
                                held += defer.pop(0)
                            if not carry:
                                while held > 0 and fillers:
                                    fillers.pop(0)()
                                    held -= 1
                    except StopIteration as si:
                        for fn in carry:
                            fn()
                        return si.value

                def share(nfill, nsteps):
                    base, rem = divmod(nfill, nsteps)
                    return [base + (1 if i < rem else 0) for i in range(nsteps)]

                def attention_unit(b, qb, fillers, carry, reserve=3,
                                   final=False):
                    npair = 2 * qb + 2
                    # hold a few fillers back to cover the close-out chain
                    # latency after the last AV pair
                    nres = min(reserve, len(fillers))
                    sh = share(len(fillers) - nres, 2 * npair)
                    close0 = weave(attention_gen(b, 0, qb), fillers, carry,
                                   defer=sh[:npair])
                    close1 = weave(attention_gen(b, 1, qb), fillers, [close0],
                                   defer=sh[npair:])
                    if final:
                        for fn in fillers:
                            fn()
                        del fillers[:]
                        close1()
                        return []
                    for fn in fillers:  # reserved + leftovers
                        fn()
                    del fillers[:]
                    return [close1]

                def prefix(gen, n):
                    for _ in range(n):
                        next(gen)
                        yield

                # ============ main schedule ============
                carry = []
                for b in range(B):
                    # ---- phase 1 (+ second half of prev batch qb3 proj) ----
                    if b > 0:
                        ph1_fill = proj_thunks(b - 1, 3, o["pat_p1"])[8:]
                        qb0_fill = proj_thunks(b - 1, 3, o["pat_attn"][3])[:8]
                    else:
                        ph1_fill, qb0_fill = [], []
                    for nb in range(QB - 1):
                        weave(phase1_nb(b, nb), ph1_fill, carry,
                              defer=share(2, KP + 1) if ph1_fill else None)
                        carry = []
                    # nb3: QK sweep + V01; the V2/V3 tail becomes PE filler
                    # for the attention(qb0) unit
                    g_ph = phase1_nb(b, 3)
                    weave(prefix(g_ph, 9), ph1_fill, carry,
                          defer=share(2, 10) if ph1_fill else None)
                    carry = []
                    for fn in ph1_fill:
                        fn()
                    tails = [(lambda: next(g_ph, None)) for _ in range(10)]
                    # ---- qb0 unit merged with phase-1 V tail ----
                    # one V2 step first fixes the O-slot rotation so the V3
                    # sweep can safely cover the h0 close-out chain
                    tails.pop(0)()
                    close0 = weave(attention_gen(b, 0, 0), tails, carry,
                                   defer=[2, 2])
                    for _ in range(3):  # V3 progress covers the dn/recip chain
                        if tails:
                            tails.pop(0)()
                    close0()
                    fill2 = tails + qb0_fill
                    close1 = weave(attention_gen(b, 1, 0), fill2,
                                   [], defer=[2, 2])
                    for fn in fill2:
                        fn()
                    carry = [close1]
                    carry = attention_unit(
                        b, 1, proj_thunks(b, 0, o["pat_attn"][0]), carry)
                    carry = attention_unit(
                        b, 2, proj_thunks(b, 1, o["pat_attn"][1]), carry)
                    carry = attention_unit(
                        b, 3, proj_thunks(b, 2, o["pat_attn"][2]
                                          if b < B - 1
                                          else "DADADADADA" + "A" * 6), carry,
                        reserve=6, final=(b == B - 1))
                # ---- tail: close-out then final batch qb3 proj ----
                for c in carry:
                    c()
                carry = []
                for fn in proj_thunks(B - 1, 3, o["pat_tail"],
                                      narrow_dma=False, tags="YO"):
                    fn()
                if debug:
                    for h in range(HPC):
                        nc.sync.dma_start(dbg["QT"][h], QTp[h][:])
                        nc.sync.dma_start(dbg["KT"][h], KTp[h][:])
                        nc.sync.dma_start(dbg["outT"][h], outTp[h][:])
                    for kt in range(NT):
                        nc.sync.dma_start(dbg["V"][kt], Vp[kt][:])

    nc.compile()
    return nc


def prep_inputs(x, w_qkv, w_o):
    """Host-side shard prep. Returns per-core input maps (bf16)."""
    bf = mybir.dt.np(BF16)
    x = np.asarray(x, dtype=np.float32).reshape(B * T, D)
    xT = np.ascontiguousarray(x.T).reshape(KC, P, B * T).astype(bf)
    w_qkv = np.asarray(w_qkv, dtype=np.float32)
    w_o = np.asarray(w_o, dtype=np.float32)

    tri = np.zeros((P, P), dtype=np.float32)
    kp = np.arange(P)[:, None]
    qu = np.arange(P)[None, :]
    tri[kp <= qu] = 1.0
    tri = tri.astype(bf)
    onr = np.ones((1, P), dtype=np.float32)

    in_maps = []
    for c in range(NCORES):
        h0, h1 = HPC * c, HPC * c + 1
        cols = []
        for h in (h0, h1):
            cols += [w_qkv[h * DK:(h + 1) * DK],            # Q rows
                     w_qkv[D + h * DK:D + (h + 1) * DK]]    # K rows
        # reorder to Q0 K0 Q1 K1 then V0 V1
        cols = [cols[0], cols[1], cols[2], cols[3],
                w_qkv[2 * D + h0 * DK:2 * D + (h0 + 1) * DK],
                w_qkv[2 * D + h1 * DK:2 * D + (h1 + 1) * DK]]
        w = np.ascontiguousarray(
            np.concatenate(cols, 0).T).reshape(KC, P, WC).astype(bf)
        wo = np.ascontiguousarray(
            w_o[:, HPC * DK * c:HPC * DK * (c + 1)].T).astype(bf)
        in_maps.append({
            "xT": xT, "w": w, "woT": wo, "tri": tri, "onr": onr,
        })
    return in_maps


_nc_cache = {}


def get_nc(debug=False, **opts):
    key = (debug, tuple(sorted((k, str(v)) for k, v in opts.items())))
    if key not in _nc_cache:
        _nc_cache[key] = build(debug=debug, **opts)
    return _nc_cache[key]


def run(x, w_qkv, w_o, debug=False, **opts):
    nc = get_nc(debug=debug, **opts)
    in_maps = prep_inputs(x, w_qkv, w_o)
    res = bass_utils.run_bass_kernel_spmd(nc, in_maps, core_ids=list(range(NCORES)))
    return res


def kernel(x, w_qkv, w_o):
    res = run(x, w_qkv, w_o)
    y = res.results[0]["y"].astype(np.float64)
    for c in range(1, NCORES):
        y += res.results[c]["y"]
    return y.astype(np.float32).reshape(B, T, D)


# revision 40
# speedup vs baseline: 1.2783x; 1.0022x over previous
"""Trainium2 Bass kernel for causal multi-head attention (dense transformer block).

Math (reference semantics):
    qkv = x @ w_qkv.T ; split into Q,K,V heads [B,H,T,dk]
    (rotary in the reference rotates Q and K of head h by a constant,
     time-independent orthogonal rotation R_h; since scores = (R_h q)·(R_h k)
     = q·k, the rotation cancels exactly and is skipped here)
    scores = causal_mask(Q @ K.T / sqrt(dk)); attn = softmax(scores)
    out = attn @ V ; y = out @ w_o.T

Sharding: head-parallel over 8 cores (2 heads/core, both batches).  Each core
computes a partial y (its heads' contribution through w_o columns); the host
sums the 8 partials (the "all-reduce").

v3 design (vs the f32r baseline):
  * All matmul inputs bf16 (same 1 cyc/row PE rate as f32r at wide free dims,
    half the DMA + SBUF).  PSUM accumulation stays fp32.
  * Phase 1 is k-outer: 6 concurrent PSUM groups (Q/K for 2 heads in two
    2-bank "S" tiles, V written DIRECTLY in [token, dk] layout into "O"
    tiles) so PE starts as soon as the first weight/x chunk lands and no
    V^T->V transposes are needed.  PSUM drains on ACT/DVE (GpSimd cannot
    touch PSUM), chunk-PAIR DMAs halve HWDGE descriptor-queue pressure.
  * Softmax denominator: bf16 pair-add (DVE 4x mode) + fp32 running sums
    split into two chains (GpSimd + DVE), then two GpSimd cross-partition
    (axis=C) reduces - no [1,512] ones-matmuls on PE (saves ~34us PE).
  * Causal narrowing: diagonal key-tiles only compute the live q-suffix in
    scores/AV/exp; the dead ex prefix is zeroed by a GpSimd memset; the
    128x128 causal triangle is masked by a GpSimd multiply.
  * proj(qb) units are woven between the attention kt-pairs of the next
    unit (qb3 into the next batch's phase 1 + qb0 unit) so the exp-gated
    stretches of attention get PE filler; yt PSUM->SBUF copies alternate
    ACT/DVE; y stores go out as one wide DMA per token tile.
  * softmax close-out chains (reduce -> recip -> broadcast-mm -> normalize)
    are deferred into the following instruction stream so PE (in-order)
    never waits on them.
"""

import contextlib

import numpy as np

import concourse.bacc as bacc
import concourse.bass as bass
import concourse.mybir as mybir
import concourse.tile as tile
from concourse import bass_utils

B, T, D, H, DK = 2, 2048, 2048, 16, 128
NCORES = 8
HPC = H // NCORES  # heads per core
P = 128
NB = 512           # q-block / token-block / e-block width
KC = D // P        # 16 contraction chunks of the model dim
KP = KC // 2       # chunk pairs
QB = T // NB       # 4 q blocks per batch
NT = T // P        # 16 token tiles per batch
WC = 6 * P         # w columns per chunk: Q0 K0 Q1 K1 V0 V1
FP32 = mybir.dt.float32
F32R = mybir.dt.float32r
BF16 = mybir.dt.bfloat16
SCALE = 1.0 / np.sqrt(DK)

DEFAULT_OPTS = dict(
    ex_bufs=4, xt_bufs=2, yt_bufs=3, s_bufs=2, o_bufs=2, y_bufs=2,
    loop_n=1,
    # yt-copy engine cycle per *hosting location* of the proj units
    pat_attn={0: "DA", 1: "DA", 2: "DA", 3: "DA"},
    pat_p1="DA", pat_tail="AD",
    defer_pairs=2,
    chain_pat={0: "PD", 1: "PD", 2: "PD", 3: "DD"},  # per qb (even, odd pair)
    mask_eng="D",
    qkv_q="A", qkv_k="D", qkv_v="AD", pipe=True, chain_bf16=True,
)

_ENG_MAP = {"P": "gpsimd", "A": "scalar", "D": "vector"}


def _width(kt, qb):
    """Live q-suffix width of key tile kt within q-block qb (causal)."""
    j = kt - 4 * qb
    if j <= 0:
        return NB
    return NB - P * j


def build(debug=False, **opts):
    o = dict(DEFAULT_OPTS)
    o.update({k: v for k, v in opts.items() if k in DEFAULT_OPTS})
    nc = bacc.Bacc("TRN2", target_bir_lowering=False, debug=False,
                   num_devices=NCORES)
    # 3D dram layouts allow one DMA per chunk-pair / token tile
    xT = nc.dram_tensor("xT", [KC, P, B * T], BF16, kind="ExternalInput")
    w_d = nc.dram_tensor("w", [KC, P, WC], BF16, kind="ExternalInput")
    woT = nc.dram_tensor("woT", [HPC * DK, D], BF16, kind="ExternalInput")
    tri_d = nc.dram_tensor("tri", [P, P], BF16, kind="ExternalInput")
    onr_d = nc.dram_tensor("onr", [1, P], F32R, kind="ExternalInput")
    y = nc.dram_tensor("y", [B * T, D], BF16, kind="ExternalOutput")
    dbg = {}
    if debug:
        for nm in ("QT", "KT", "outT"):
            dbg[nm] = nc.dram_tensor(f"dbg_{nm}", [HPC, P, T], BF16,
                                     kind="ExternalOutput")
        dbg["V"] = nc.dram_tensor("dbg_V", [NT, P, 2 * P], BF16,
                                  kind="ExternalOutput")

    with tile.TileContext(nc) as tc:
        with (
            tc.tile_pool(name="const", bufs=1) as cpool,
            tc.tile_pool(name="xp", bufs=1) as xpool,
            tc.tile_pool(name="qkv", bufs=1) as qpool,
            tc.tile_pool(name="attn", bufs=1) as apool,
            tc.tile_pool(name="ps", bufs=1, space="PSUM") as pspool,
        ):
            # ---- constants / weights resident in SBUF ----
            w_sb = [cpool.tile([P, 2, WC], BF16, name=f"w_{kp}") for kp in range(KP)]
            wo_sb = [cpool.tile([P, D], BF16, name=f"wo_{h}") for h in range(HPC)]
            tri = cpool.tile([P, P], BF16, name="tri")
            onr = cpool.tile([1, P], F32R, name="onr")

            def wqk(k, m):  # m in 0..3 = Q0 K0 Q1 K1 of chunk k
                return w_sb[k // 2][:, k % 2, m * P:(m + 1) * P]

            def wvv(k):     # V columns (both heads) of chunk k
                return w_sb[k // 2][:, k % 2, 4 * P:6 * P]

            # persistent per-batch state (WAR deps recycle across batches)
            QTp = [qpool.tile([P, T], BF16, name=f"QT{h}") for h in range(HPC)]
            KTp = [qpool.tile([P, T], BF16, name=f"KT{h}") for h in range(HPC)]
            Vp = [qpool.tile([P, 2 * P], BF16, name=f"V{kt}") for kt in range(NT)]
            outTp = [qpool.tile([P, T], BF16, name=f"outT{h}") for h in range(HPC)]

            def ps_tile(tag, shape, name, bufs):
                return pspool.tile(shape, FP32, name=name, tag=tag, bufs=bufs)

            def copy_on(code, dst, src):
                eng = _ENG_MAP[code]
                if eng == "gpsimd":
                    nc.gpsimd.tensor_copy(dst, src)
                elif eng == "scalar":
                    nc.scalar.copy(dst, src)
                else:
                    nc.vector.tensor_copy(dst, src)

            loop_ctx = (tc.For_i(0, o["loop_n"], 1, hint_engines=(
                            mybir.EngineType.PE, mybir.EngineType.Activation,
                            mybir.EngineType.DVE, mybir.EngineType.SP,
                            mybir.EngineType.Pool))
                        if o["loop_n"] > 1 else contextlib.nullcontext())

            if o["loop_n"] > 1:
                # weights/constants loaded once, outside the HW loop
                for kp in range(KP):
                    nc.sync.dma_start(w_sb[kp][:], w_d[2 * kp:2 * kp + 2])
                nc.sync.dma_start(tri[:], tri_d[:, :])
                nc.sync.dma_start(onr[:], onr_d[:, :])
                for h in range(HPC):
                    nc.sync.dma_start(wo_sb[h][:], woT[h * P:(h + 1) * P, :])

            with loop_ctx:
                # ============ phase 1 generator (one token block) ============
                def load_nb(b, nb):
                    """Allocate + DMA the x tiles for token block (b, nb)."""
                    col0 = b * T + nb * NB
                    xt = [xpool.tile([P, 2, NB], BF16, name=f"x{kp}_{b}_{nb}",
                                     tag=f"x{kp}", bufs=o["xt_bufs"])
                          for kp in range(KP)]
                    first = b == 0 and nb == 0 and o["loop_n"] == 1
                    for kp in range(KP):
                        if first and kp == 0:
                            # column-split: first QK slice lands sooner (row
                            # interleave of the pair DMA is preserved)
                            nc.sync.dma_start(w_sb[0][:, :, 0:2 * P],
                                              w_d[0:2, :, 0:2 * P])
                            nc.sync.dma_start(w_sb[0][:, :, 2 * P:WC],
                                              w_d[0:2, :, 2 * P:WC])
                        elif first:
                            nc.sync.dma_start(w_sb[kp][:], w_d[2 * kp:2 * kp + 2])
                        nc.sync.dma_start(xt[kp][:],
                                          xT[2 * kp:2 * kp + 2, :, col0:col0 + NB])
                        if first and kp == 0:
                            nc.sync.dma_start(tri[:], tri_d[:, :])
                            nc.sync.dma_start(onr[:], onr_d[:, :])
                    return xt

                def phase1_nb(b, nb, xt, mid=None):
                    """QKV projection for token block (b, nb), k-outer.
                    Yields after each chunk-pair (8) + drains; `mid` thunk
                    (next-block prefetch) fires after chunk-pair 5."""
                    if b == 0 and nb == 1 and o["loop_n"] == 1:
                        for h in range(HPC):
                            nc.sync.dma_start(wo_sb[h][:], woT[h * P:(h + 1) * P, :])

                    S0 = ps_tile("S", [P, 2, NB], f"p1s0_{b}_{nb}", o["s_bufs"])
                    S1 = ps_tile("S", [P, 2, NB], f"p1s1_{b}_{nb}", o["s_bufs"])
                    # V token-tile groups need a PSUM bank each (one
                    # accumulation group per bank): two sub-sweeps of 2.
                    V01 = [ps_tile("O", [P, NB], f"p1v{t}_{b}_{nb}", o["o_bufs"])
                           for t in range(2)]
                    for kp in range(KP):
                        for half in range(2):
                            k = 2 * kp + half
                            st, sp = k == 0, k == KC - 1
                            xk = xt[kp][:, half, :]
                            nc.tensor.matmul(S0[:, 0, :], wqk(k, 0), xk,
                                             start=st, stop=sp)
                            nc.tensor.matmul(S0[:, 1, :], wqk(k, 1), xk,
                                             start=st, stop=sp)
                            nc.tensor.matmul(S1[:, 0, :], wqk(k, 2), xk,
                                             start=st, stop=sp)
                            nc.tensor.matmul(S1[:, 1, :], wqk(k, 3), xk,
                                             start=st, stop=sp)
                            for t in range(2):
                                nc.tensor.matmul(V01[t][:, 0:2 * P],
                                                 xt[kp][:, half, t * P:(t + 1) * P],
                                                 wvv(k), start=st, stop=sp)
                        if kp == 5 and mid is not None:
                            mid()
                        yield
                    # drain QK + first V pair; second V pair sweep follows
                    csl = slice(nb * NB, (nb + 1) * NB)
                    copy_on(o["qkv_q"], QTp[0][:, csl], S0[:, 0, :])
                    copy_on(o["qkv_k"], KTp[0][:, csl], S0[:, 1, :])
                    copy_on(o["qkv_q"], QTp[1][:, csl], S1[:, 0, :])
                    copy_on(o["qkv_k"], KTp[1][:, csl], S1[:, 1, :])
                    for t in range(2):
                        copy_on(o["qkv_v"][t % len(o["qkv_v"])],
                                Vp[nb * 4 + t][:], V01[t][:, 0:2 * P])
                    yield
                    # V2 then V3 sequentially: each holds only ONE O slot,
                    # so attention(qb0) can interleave using the other slot
                    for t in range(2, 4):
                        Vt = ps_tile("O", [P, NB], f"p1v{t}_{b}_{nb}",
                                     o["o_bufs"])
                        for kp in range(KP):
                            for half in range(2):
                                k = 2 * kp + half
                                nc.tensor.matmul(Vt[:, 0:2 * P],
                                                 xt[kp][:, half,
                                                        t * P:(t + 1) * P],
                                                 wvv(k), start=(k == 0),
                                                 stop=(k == KC - 1))
                            if kp % 2 == 1:
                                yield
                        copy_on(o["qkv_v"][t % len(o["qkv_v"])],
                                Vp[nb * 4 + t][:], Vt[:, 0:2 * P])
                        yield

                # ============ attention generator (one head) ============
                def attention_gen(b, h, qb):
                    """Yields once per kt-pair.  Returns the deferred
                    close-out thunk (bcmm + normalize)."""
                    nkt = 4 * qb + 4
                    qsl0 = qb * NB
                    ps_o = ps_tile("O", [P, NB], f"pso_{b}_{h}_{qb}", o["o_bufs"])
                    # two running-sum chains: even pairs / odd pairs
                    cdt = BF16 if o["chain_bf16"] else FP32
                    exs = [apool.tile([P, NB], cdt, name=f"exs{i}_{b}_{h}_{qb}",
                                      tag=f"exsum{i}", bufs=2) for i in range(2)]
                    npair = nkt // 2

                    def emit_scores_exp(p):
                        a, c = 2 * p, 2 * p + 1
                        oa, oc = NB - _width(a, qb), NB - _width(c, qb)
                        ps_s = ps_tile("S", [P, 2, NB], f"pss_{b}_{h}_{qb}_{p}",
                                       o["s_bufs"])
                        nc.tensor.matmul(ps_s[:, 0, oa:NB],
                                         KTp[h][:, a * P:(a + 1) * P],
                                         QTp[h][:, qsl0 + oa:qsl0 + NB],
                                         start=True, stop=True)
                        nc.tensor.matmul(ps_s[:, 1, oc:NB],
                                         KTp[h][:, c * P:(c + 1) * P],
                                         QTp[h][:, qsl0 + oc:qsl0 + NB],
                                         start=True, stop=True)
                        ex = apool.tile([P, 2, NB], BF16,
                                        name=f"ex_{b}_{h}_{qb}_{p}",
                                        tag="ex", bufs=o["ex_bufs"])
                        if oa == oc:
                            nc.scalar.activation(ex[:, :, oa:NB],
                                                 ps_s[:, :, oa:NB],
                                                 mybir.ActivationFunctionType.Exp,
                                                 scale=SCALE)
                        else:
                            nc.scalar.activation(ex[:, 0, oa:NB],
                                                 ps_s[:, 0, oa:NB],
                                                 mybir.ActivationFunctionType.Exp,
                                                 scale=SCALE)
                            nc.scalar.activation(ex[:, 1, oc:NB],
                                                 ps_s[:, 1, oc:NB],
                                                 mybir.ActivationFunctionType.Exp,
                                                 scale=SCALE)
                        return (p, ex, oa, oc)

                    def emit_post(st):
                        p, ex, oa, oc = st
                        a, c = 2 * p, 2 * p + 1
                        # zero dead prefixes of narrowed (diagonal) tiles
                        if oa > 0:
                            nc.gpsimd.memset(ex[:, 0, 0:oa], 0.0)
                        if oc > 0:
                            nc.gpsimd.memset(ex[:, 1, 0:oc], 0.0)
                        # triangle masks on diagonal tiles
                        for half, kt, off in ((0, a, oa), (1, c, oc)):
                            if kt >= 4 * qb:
                                sl = ex[:, half, off:off + P]
                                if o["mask_eng"] == "P":
                                    nc.gpsimd.tensor_mul(sl, sl, tri[:])
                                else:
                                    nc.vector.tensor_mul(sl, sl, tri[:])
                        # denominator partial: exs[p%2] += ex.lo + ex.hi
                        tpr = apool.tile([P, NB], BF16,
                                         name=f"tp_{b}_{h}_{qb}_{p}",
                                         tag="tpr", bufs=2)
                        nc.vector.tensor_add(tpr[:], ex[:, 0, :], ex[:, 1, :])
                        cp = o["chain_pat"][qb] if isinstance(o["chain_pat"], dict) else o["chain_pat"]
                        eng = getattr(nc, _ENG_MAP[cp[p % 2]])
                        if p < 2:
                            eng.tensor_copy(exs[p % 2][:], tpr[:])
                        else:
                            eng.tensor_add(exs[p % 2][:], exs[p % 2][:], tpr[:])
                        # AV accumulation
                        nc.tensor.matmul(ps_o[:, oa:NB],
                                         Vp[a][:, h * P:(h + 1) * P],
                                         ex[:, 0, oa:NB],
                                         start=(p == 0), stop=False,
                                         skip_group_check=True)
                        nc.tensor.matmul(ps_o[:, oc:NB],
                                         Vp[c][:, h * P:(h + 1) * P],
                                         ex[:, 1, oc:NB],
                                         start=False, stop=(p == npair - 1),
                                         skip_group_check=True)

                    # software pipeline: scores/exp of p+1 before AV of p
                    if o["pipe"]:
                        st = emit_scores_exp(0)
                        for p in range(npair):
                            nxt = (emit_scores_exp(p + 1)
                                   if p + 1 < npair else None)
                            emit_post(st)
                            st = nxt
                            yield
                    else:
                        for p in range(npair):
                            emit_post(emit_scores_exp(p))
                            yield
                    # denominator: merge chains, cross-partition reduce, recip
                    rec = apool.tile([1, NB], F32R, name=f"rec_{b}_{h}_{qb}",
                                     tag="rec", bufs=2)
                    dn = apool.tile([1, NB], FP32, name=f"dn_{b}_{h}_{qb}",
                                    tag="dn", bufs=2)
                    if o["chain_bf16"]:
                        mrg = apool.tile([P, NB], BF16, name=f"mg_{b}_{h}_{qb}",
                                         tag="mrg", bufs=2)
                        nc.vector.tensor_add(mrg[:], exs[0][:], exs[1][:])
                        nc.gpsimd.tensor_reduce(dn[:1, :], mrg[:],
                                                axis=mybir.AxisListType.C,
                                                op=mybir.AluOpType.add)
                    else:
                        dn1 = apool.tile([1, NB], FP32, name=f"dn1_{b}_{h}_{qb}",
                                         tag="dn1", bufs=2)
                        nc.gpsimd.tensor_reduce(dn[:1, :], exs[0][:],
                                                axis=mybir.AxisListType.C,
                                                op=mybir.AluOpType.add)
                        nc.gpsimd.tensor_reduce(dn1[:1, :], exs[1][:],
                                                axis=mybir.AxisListType.C,
                                                op=mybir.AluOpType.add)
                        nc.vector.tensor_add(dn[:1, :], dn[:1, :], dn1[:1, :])
                    with nc.allow_low_precision(reason="f32r recip: tf32 ok"):
                        nc.vector.reciprocal(rec[:1, :], dn[:1, :])

                    def close():
                        ps_bc = ps_tile("Y", [P, NB], f"psbc_{b}_{h}_{qb}",
                                        o["y_bufs"])
                        nc.tensor.matmul(ps_bc[:], onr[:1, :], rec[:1, :],
                                         start=True, stop=True)
                        bc = apool.tile([P, NB], FP32, name=f"bc_{b}_{h}_{qb}",
                                        tag="bc", bufs=2)
                        nc.vector.tensor_copy(bc[:], ps_bc[:])
                        nc.vector.tensor_mul(outTp[h][:, qsl0:qsl0 + NB],
                                             ps_o[:], bc[:])
                    return close

                # ============ proj units ============
                yts = {}

                def proj_unit(b, tt, eb, eng_code, narrow_dma=False,
                              tag="Y"):
                    ps_y = ps_tile(tag, [P, NB], f"psy_{b}_{tt}_{eb}",
                                   o["y_bufs"])
                    nc.tensor.matmul(ps_y[:], outTp[0][:, tt * P:(tt + 1) * P],
                                     wo_sb[0][:, eb * NB:(eb + 1) * NB],
                                     start=True, stop=False)
                    nc.tensor.matmul(ps_y[:], outTp[1][:, tt * P:(tt + 1) * P],
                                     wo_sb[1][:, eb * NB:(eb + 1) * NB],
                                     start=False, stop=True)
                    if eb == 0:
                        yts[(b, tt)] = apool.tile([P, QB, NB], BF16,
                                                  name=f"yt_{b}_{tt}",
                                                  tag="yt", bufs=o["yt_bufs"])
                    yt = yts[(b, tt)]
                    copy_on(eng_code, yt[:, eb, :], ps_y[:])
                    if narrow_dma:
                        # tail: stream each e-block out as soon as copied
                        nc.sync.dma_start(
                            y[b * T + tt * P:b * T + (tt + 1) * P,
                              eb * NB:(eb + 1) * NB], yt[:, eb, :])
                    elif eb == QB - 1:
                        nc.sync.dma_start(
                            y[b * T + tt * P:b * T + (tt + 1) * P, :], yt[:])

                def proj_thunks(b, pqb, pat, narrow_dma=False, tags="Y"):
                    th = []
                    i = 0
                    for tt in range(4 * pqb, 4 * pqb + 4):
                        for eb in range(QB):
                            code = pat[i % len(pat)]
                            tag = tags[i % len(tags)]
                            th.append(lambda b=b, tt=tt, eb=eb, code=code,
                                      tag=tag:
                                      proj_unit(b, tt, eb, code, narrow_dma,
                                                tag))
                            i += 1
                    return th

                # ============ weaving driver ============
                def weave(gen, fillers, carry, defer=None):
                    """Run gen; after each yield emit carry thunks (once,
                    after o['defer_pairs'] yields) and a fair share of
                    fillers (popped from the shared list)."""
                    n = 0
                    held = 0
                    try:
                        while True:
                            next(gen)
                            n += 1
                            if n >= o["defer_pairs"] and carry:
                                for fn in carry:
                                    fn()
                                carry = []
                            if defer:
                                held += defer.pop(0)
                            if not carry:
                                while held > 0 and fillers:
                                    fillers.pop(0)()
                                    held -= 1
                    except StopIteration as si:
                        for fn in carry:
                            fn()
                        return si.value

                def share(nfill, nsteps):
                    base, rem = divmod(nfill, nsteps)
                    return [base + (1 if i < rem else 0) for i in range(nsteps)]

                def attention_unit(b, qb, fillers, carry, reserve=3,
                                   final=False):
                    npair = 2 * qb + 2
                    # hold a few fillers back to cover the close-out chain
                    # latency after the last AV pair
                    nres = min(reserve, len(fillers))
                    sh = share(len(fillers) - nres, 2 * npair)
                    close0 = weave(attention_gen(b, 0, qb), fillers, carry,
                                   defer=sh[:npair])
                    close1 = weave(attention_gen(b, 1, qb), fillers, [close0],
                                   defer=sh[npair:])
                    if final:
                        for fn in fillers:
                            fn()
                        del fillers[:]
                        close1()
                        return []
                    for fn in fillers:  # reserved + leftovers
                        fn()
                    del fillers[:]
                    return [close1]

                def prefix(gen, n):
                    for _ in range(n):
                        next(gen)
                        yield

                # ============ main schedule ============
                carry = []
                xts = {}
                for b in range(B):
                    # ---- phase 1 (+ second half of prev batch qb3 proj) ----
                    if b > 0:
                        ph1_fill = proj_thunks(b - 1, 3, o["pat_p1"])[8:]
                        qb0_fill = proj_thunks(b - 1, 3, o["pat_attn"][3])[:8]
                    else:
                        ph1_fill, qb0_fill = [], []
                    if b == 0:
                        xts["cur"] = load_nb(0, 0)
                    for nb in range(QB - 1):
                        nxt = [b, nb + 1]
                        mid = (lambda nxt=nxt:
                               xts.__setitem__("next", load_nb(*nxt)))
                        weave(phase1_nb(b, nb, xts["cur"], mid), ph1_fill,
                              carry,
                              defer=share(2, KP + 1) if ph1_fill else None)
                        xts["cur"] = xts["next"]
                        carry = []
                    # nb3: QK sweep + V01; the V2/V3 tail becomes PE filler
                    # for the attention(qb0) unit
                    mid = ((lambda: xts.__setitem__("next", load_nb(b + 1, 0)))
                           if b + 1 < B else None)
                    g_ph = phase1_nb(b, 3, xts["cur"], mid)
                    weave(prefix(g_ph, 9), ph1_fill, carry,
                          defer=share(2, 10) if ph1_fill else None)
                    if b + 1 < B:
                        xts["cur"] = xts["next"]
                    carry = []
                    for fn in ph1_fill:
                        fn()
                    tails = [(lambda: next(g_ph, None)) for _ in range(10)]
                    # ---- qb0 unit merged with phase-1 V tail ----
                    # one V2 step first fixes the O-slot rotation so the V3
                    # sweep can safely cover the h0 close-out chain
                    tails.pop(0)()
                    close0 = weave(attention_gen(b, 0, 0), tails, carry,
                                   defer=[2, 2])
                    for _ in range(3):  # V3 progress covers the dn/recip chain
                        if tails:
                            tails.pop(0)()
                    close0()
                    fill2 = tails + qb0_fill
                    close1 = weave(attention_gen(b, 1, 0), fill2,
                                   [], defer=[2, 2])
                    for fn in fill2:
                        fn()
                    carry = [close1]
                    carry = attention_unit(
                        b, 1, proj_thunks(b, 0, o["pat_attn"][0]), carry)
                    carry = attention_unit(
                        b, 2, proj_thunks(b, 1, o["pat_attn"][1]), carry)
                    carry = attention_unit(
                        b, 3, proj_thunks(b, 2, o["pat_attn"][2]
                                          if b < B - 1
                                          else "DADADADADA" + "A" * 6), carry,
                        reserve=6, final=(b == B - 1))
                # ---- tail: close-out then final batch qb3 proj ----
                for c in carry:
                    c()
                carry = []
                for fn in proj_thunks(B - 1, 3, o["pat_tail"],
                                      narrow_dma=False, tags="YO"):
                    fn()
                if debug:
                    for h in range(HPC):
                        nc.sync.dma_start(dbg["QT"][h], QTp[h][:])
                        nc.sync.dma_start(dbg["KT"][h], KTp[h][:])
                        nc.sync.dma_start(dbg["outT"][h], outTp[h][:])
                    for kt in range(NT):
                        nc.sync.dma_start(dbg["V"][kt], Vp[kt][:])

    nc.compile()
    return nc


def prep_inputs(x, w_qkv, w_o):
    """Host-side shard prep. Returns per-core input maps (bf16)."""
    bf = mybir.dt.np(BF16)
    x = np.asarray(x, dtype=np.float32).reshape(B * T, D)
    xT = np.ascontiguousarray(x.T).reshape(KC, P, B * T).astype(bf)
    w_qkv = np.asarray(w_qkv, dtype=np.float32)
    w_o = np.asarray(w_o, dtype=np.float32)

    tri = np.zeros((P, P), dtype=np.float32)
    kp = np.arange(P)[:, None]
    qu = np.arange(P)[None, :]
    tri[kp <= qu] = 1.0
    tri = tri.astype(bf)
    onr = np.ones((1, P), dtype=np.float32)

    in_maps = []
    for c in range(NCORES):
        h0, h1 = HPC * c, HPC * c + 1
        cols = []
        for h in (h0, h1):
            cols += [w_qkv[h * DK:(h + 1) * DK],            # Q rows
                     w_qkv[D + h * DK:D + (h + 1) * DK]]    # K rows
        # reorder to Q0 K0 Q1 K1 then V0 V1
        cols = [cols[0], cols[1], cols[2], cols[3],
                w_qkv[2 * D + h0 * DK:2 * D + (h0 + 1) * DK],
                w_qkv[2 * D + h1 * DK:2 * D + (h1 + 1) * DK]]
        w = np.ascontiguousarray(
            np.concatenate(cols, 0).T).reshape(KC, P, WC).astype(bf)
        wo = np.ascontiguousarray(
            w_o[:, HPC * DK * c:HPC * DK * (c + 1)].T).astype(bf)
        in_maps.append({
            "xT": xT, "w": w, "woT": wo, "tri": tri, "onr": onr,
        })
    return in_maps


_nc_cache = {}


def get_nc(debug=False, **opts):
    key = (debug, tuple(sorted((k, str(v)) for k, v in opts.items())))
    if key not in _nc_cache:
        _nc_cache[key] = build(debug=debug, **opts)
    return _nc_cache[key]


def run(x, w_qkv, w_o, debug=False, **opts):
    nc = get_nc(debug=debug, **opts)
    in_maps = prep_inputs(x, w_qkv, w_o)
    res = bass_utils.run_bass_kernel_spmd(nc, in_maps, core_ids=list(range(NCORES)))
    return res


def kernel(x, w_qkv, w_o):
    res = run(x, w_qkv, w_o)
    y = res.results[0]["y"].astype(np.float64)
    for c in range(1, NCORES):
        y += res.results[c]["y"]
    return y.astype(np.float32).reshape(B, T, D)


# revision 41
# speedup vs baseline: 1.2791x; 1.0006x over previous
"""Trainium2 Bass kernel for causal multi-head attention (dense transformer block).

Math (reference semantics):
    qkv = x @ w_qkv.T ; split into Q,K,V heads [B,H,T,dk]
    (rotary in the reference rotates Q and K of head h by a constant,
     time-independent orthogonal rotation R_h; since scores = (R_h q)·(R_h k)
     = q·k, the rotation cancels exactly and is skipped here)
    scores = causal_mask(Q @ K.T / sqrt(dk)); attn = softmax(scores)
    out = attn @ V ; y = out @ w_o.T

Sharding: head-parallel over 8 cores (2 heads/core, both batches).  Each core
computes a partial y (its heads' contribution through w_o columns); the host
sums the 8 partials (the "all-reduce").

v3 design (vs the f32r baseline):
  * All matmul inputs bf16 (same 1 cyc/row PE rate as f32r at wide free dims,
    half the DMA + SBUF).  PSUM accumulation stays fp32.
  * Phase 1 is k-outer: 6 concurrent PSUM groups (Q/K for 2 heads in two
    2-bank "S" tiles, V written DIRECTLY in [token, dk] layout into "O"
    tiles) so PE starts as soon as the first weight/x chunk lands and no
    V^T->V transposes are needed.  PSUM drains on ACT/DVE (GpSimd cannot
    touch PSUM), chunk-PAIR DMAs halve HWDGE descriptor-queue pressure.
  * Softmax denominator: bf16 pair-add (DVE 4x mode) + fp32 running sums
    split into two chains (GpSimd + DVE), then two GpSimd cross-partition
    (axis=C) reduces - no [1,512] ones-matmuls on PE (saves ~34us PE).
  * Causal narrowing: diagonal key-tiles only compute the live q-suffix in
    scores/AV/exp; the dead ex prefix is zeroed by a GpSimd memset; the
    128x128 causal triangle is masked by a GpSimd multiply.
  * proj(qb) units are woven between the attention kt-pairs of the next
    unit (qb3 into the next batch's phase 1 + qb0 unit) so the exp-gated
    stretches of attention get PE filler; yt PSUM->SBUF copies alternate
    ACT/DVE; y stores go out as one wide DMA per token tile.
  * softmax close-out chains (reduce -> recip -> broadcast-mm -> normalize)
    are deferred into the following instruction stream so PE (in-order)
    never waits on them.
"""

import contextlib

import numpy as np

import concourse.bacc as bacc
import concourse.bass as bass
import concourse.mybir as mybir
import concourse.tile as tile
from concourse import bass_utils

B, T, D, H, DK = 2, 2048, 2048, 16, 128
NCORES = 8
HPC = H // NCORES  # heads per core
P = 128
NB = 512           # q-block / token-block / e-block width
KC = D // P        # 16 contraction chunks of the model dim
KP = KC // 2       # chunk pairs
QB = T // NB       # 4 q blocks per batch
NT = T // P        # 16 token tiles per batch
WC = 6 * P         # w columns per chunk: Q0 K0 Q1 K1 V0 V1
FP32 = mybir.dt.float32
F32R = mybir.dt.float32r
BF16 = mybir.dt.bfloat16
SCALE = 1.0 / np.sqrt(DK)

DEFAULT_OPTS = dict(
    ex_bufs=4, xt_bufs=2, yt_bufs=3, s_bufs=2, o_bufs=2, y_bufs=2,
    loop_n=1,
    # yt-copy engine cycle per *hosting location* of the proj units
    pat_attn={0: "DA", 1: "DA", 2: "DA", 3: "DA"},
    pat_p1="DA", pat_tail="AD",
    defer_pairs=2,
    chain_pat={0: "PD", 1: "PD", 2: "PD", 3: "DD"},  # per qb (even, odd pair)
    mask_eng="D",
    qkv_q="A", qkv_k="D", qkv_v="AD", pipe=True, chain_bf16=True,
)

_ENG_MAP = {"P": "gpsimd", "A": "scalar", "D": "vector"}


def _width(kt, qb):
    """Live q-suffix width of key tile kt within q-block qb (causal)."""
    j = kt - 4 * qb
    if j <= 0:
        return NB
    return NB - P * j


def build(debug=False, **opts):
    o = dict(DEFAULT_OPTS)
    o.update({k: v for k, v in opts.items() if k in DEFAULT_OPTS})
    nc = bacc.Bacc("TRN2", target_bir_lowering=False, debug=False,
                   num_devices=NCORES)
    # 3D dram layouts allow one DMA per chunk-pair / token tile
    xT = nc.dram_tensor("xT", [KC, P, B * T], BF16, kind="ExternalInput")
    w_d = nc.dram_tensor("w", [KC, P, WC], BF16, kind="ExternalInput")
    woT = nc.dram_tensor("woT", [HPC * DK, D], BF16, kind="ExternalInput")
    tri_d = nc.dram_tensor("tri", [P, P], BF16, kind="ExternalInput")
    onr_d = nc.dram_tensor("onr", [1, P], F32R, kind="ExternalInput")
    y = nc.dram_tensor("y", [B * T, D], BF16, kind="ExternalOutput")
    dbg = {}
    if debug:
        for nm in ("QT", "KT", "outT"):
            dbg[nm] = nc.dram_tensor(f"dbg_{nm}", [HPC, P, T], BF16,
                                     kind="ExternalOutput")
        dbg["V"] = nc.dram_tensor("dbg_V", [NT, P, 2 * P], BF16,
                                  kind="ExternalOutput")

    with tile.TileContext(nc) as tc:
        with (
            tc.tile_pool(name="const", bufs=1) as cpool,
            tc.tile_pool(name="xp", bufs=1) as xpool,
            tc.tile_pool(name="qkv", bufs=1) as qpool,
            tc.tile_pool(name="attn", bufs=1) as apool,
            tc.tile_pool(name="ps", bufs=1, space="PSUM") as pspool,
        ):
            # ---- constants / weights resident in SBUF ----
            w_sb = [cpool.tile([P, 2, WC], BF16, name=f"w_{kp}") for kp in range(KP)]
            wo_sb = [cpool.tile([P, D], BF16, name=f"wo_{h}") for h in range(HPC)]
            tri = cpool.tile([P, P], BF16, name="tri")
            onr = cpool.tile([1, P], F32R, name="onr")

            def wqk(k, m):  # m in 0..3 = Q0 K0 Q1 K1 of chunk k
                return w_sb[k // 2][:, k % 2, m * P:(m + 1) * P]

            def wvv(k):     # V columns (both heads) of chunk k
                return w_sb[k // 2][:, k % 2, 4 * P:6 * P]

            # persistent per-batch state (WAR deps recycle across batches)
            QTp = [qpool.tile([P, T], BF16, name=f"QT{h}") for h in range(HPC)]
            KTp = [qpool.tile([P, T], BF16, name=f"KT{h}") for h in range(HPC)]
            Vp = [qpool.tile([P, 2 * P], BF16, name=f"V{kt}") for kt in range(NT)]
            outTp = [qpool.tile([P, T], BF16, name=f"outT{h}") for h in range(HPC)]

            def ps_tile(tag, shape, name, bufs):
                return pspool.tile(shape, FP32, name=name, tag=tag, bufs=bufs)

            def copy_on(code, dst, src):
                eng = _ENG_MAP[code]
                if eng == "gpsimd":
                    nc.gpsimd.tensor_copy(dst, src)
                elif eng == "scalar":
                    nc.scalar.copy(dst, src)
                else:
                    nc.vector.tensor_copy(dst, src)

            loop_ctx = (tc.For_i(0, o["loop_n"], 1, hint_engines=(
                            mybir.EngineType.PE, mybir.EngineType.Activation,
                            mybir.EngineType.DVE, mybir.EngineType.SP,
                            mybir.EngineType.Pool))
                        if o["loop_n"] > 1 else contextlib.nullcontext())

            if o["loop_n"] > 1:
                # weights/constants loaded once, outside the HW loop
                for kp in range(KP):
                    nc.sync.dma_start(w_sb[kp][:], w_d[2 * kp:2 * kp + 2])
                nc.sync.dma_start(tri[:], tri_d[:, :])
                nc.sync.dma_start(onr[:], onr_d[:, :])
                for h in range(HPC):
                    nc.sync.dma_start(wo_sb[h][:], woT[h * P:(h + 1) * P, :])

            with loop_ctx:
                # ============ phase 1 generator (one token block) ============
                def load_nb(b, nb):
                    """Allocate + DMA the x tiles for token block (b, nb)."""
                    col0 = b * T + nb * NB
                    xt = [xpool.tile([P, 2, NB], BF16, name=f"x{kp}_{b}_{nb}",
                                     tag=f"x{kp}", bufs=o["xt_bufs"])
                          for kp in range(KP)]
                    first = b == 0 and nb == 0 and o["loop_n"] == 1
                    for kp in range(KP):
                        if first and kp == 0:
                            # column-split: first QK slice lands sooner (row
                            # interleave of the pair DMA is preserved)
                            nc.sync.dma_start(w_sb[0][:, :, 0:2 * P],
                                              w_d[0:2, :, 0:2 * P])
                            nc.sync.dma_start(w_sb[0][:, :, 2 * P:WC],
                                              w_d[0:2, :, 2 * P:WC])
                        elif first:
                            nc.sync.dma_start(w_sb[kp][:], w_d[2 * kp:2 * kp + 2])
                        nc.sync.dma_start(xt[kp][:],
                                          xT[2 * kp:2 * kp + 2, :, col0:col0 + NB])
                        if first and kp == 0:
                            nc.sync.dma_start(tri[:], tri_d[:, :])
                            nc.sync.dma_start(onr[:], onr_d[:, :])
                    return xt

                def phase1_nb(b, nb, xt, mid=None):
                    """QKV projection for token block (b, nb), k-outer.
                    Yields after each chunk-pair (8) + drains; `mid` thunk
                    (next-block prefetch) fires after chunk-pair 5."""
                    if b == 0 and nb == 1 and o["loop_n"] == 1:
                        for h in range(HPC):
                            nc.sync.dma_start(wo_sb[h][:], woT[h * P:(h + 1) * P, :])

                    S0 = ps_tile("S", [P, 2, NB], f"p1s0_{b}_{nb}", o["s_bufs"])
                    S1 = ps_tile("S", [P, 2, NB], f"p1s1_{b}_{nb}", o["s_bufs"])
                    # V token-tile groups need a PSUM bank each (one
                    # accumulation group per bank): two sub-sweeps of 2.
                    V01 = [ps_tile("O", [P, NB], f"p1v{t}_{b}_{nb}", o["o_bufs"])
                           for t in range(2)]
                    for kp in range(KP):
                        for half in range(2):
                            k = 2 * kp + half
                            st, sp = k == 0, k == KC - 1
                            xk = xt[kp][:, half, :]
                            nc.tensor.matmul(S0[:, 0, :], wqk(k, 0), xk,
                                             start=st, stop=sp)
                            nc.tensor.matmul(S0[:, 1, :], wqk(k, 1), xk,
                                             start=st, stop=sp)
                            nc.tensor.matmul(S1[:, 0, :], wqk(k, 2), xk,
                                             start=st, stop=sp)
                            nc.tensor.matmul(S1[:, 1, :], wqk(k, 3), xk,
                                             start=st, stop=sp)
                            for t in range(2):
                                nc.tensor.matmul(V01[t][:, 0:2 * P],
                                                 xt[kp][:, half, t * P:(t + 1) * P],
                                                 wvv(k), start=st, stop=sp)
                        if kp == 5 and mid is not None:
                            mid()
                        yield
                    # drain V first (V2/V3 sweeps wait on these PSUM
                    # banks), then QK (next block's S tiles are far off)
                    csl = slice(nb * NB, (nb + 1) * NB)
                    for t in range(2):
                        copy_on(o["qkv_v"][t % len(o["qkv_v"])],
                                Vp[nb * 4 + t][:], V01[t][:, 0:2 * P])
                    copy_on(o["qkv_q"], QTp[0][:, csl], S0[:, 0, :])
                    copy_on(o["qkv_k"], KTp[0][:, csl], S0[:, 1, :])
                    copy_on(o["qkv_q"], QTp[1][:, csl], S1[:, 0, :])
                    copy_on(o["qkv_k"], KTp[1][:, csl], S1[:, 1, :])
                    yield
                    # V2 then V3 sequentially: each holds only ONE O slot,
                    # so attention(qb0) can interleave using the other slot
                    for t in range(2, 4):
                        Vt = ps_tile("O", [P, NB], f"p1v{t}_{b}_{nb}",
                                     o["o_bufs"])
                        for kp in range(KP):
                            for half in range(2):
                                k = 2 * kp + half
                                nc.tensor.matmul(Vt[:, 0:2 * P],
                                                 xt[kp][:, half,
                                                        t * P:(t + 1) * P],
                                                 wvv(k), start=(k == 0),
                                                 stop=(k == KC - 1))
                            if kp % 2 == 1:
                                yield
                        copy_on(o["qkv_v"][t % len(o["qkv_v"])],
                                Vp[nb * 4 + t][:], Vt[:, 0:2 * P])
                        yield

                # ============ attention generator (one head) ============
                def attention_gen(b, h, qb):
                    """Yields once per kt-pair.  Returns the deferred
                    close-out thunk (bcmm + normalize)."""
                    nkt = 4 * qb + 4
                    qsl0 = qb * NB
                    ps_o = ps_tile("O", [P, NB], f"pso_{b}_{h}_{qb}", o["o_bufs"])
                    # two running-sum chains: even pairs / odd pairs
                    cdt = BF16 if o["chain_bf16"] else FP32
                    exs = [apool.tile([P, NB], cdt, name=f"exs{i}_{b}_{h}_{qb}",
                                      tag=f"exsum{i}", bufs=2) for i in range(2)]
                    npair = nkt // 2

                    def emit_scores_exp(p):
                        a, c = 2 * p, 2 * p + 1
                        oa, oc = NB - _width(a, qb), NB - _width(c, qb)
                        ps_s = ps_tile("S", [P, 2, NB], f"pss_{b}_{h}_{qb}_{p}",
                                       o["s_bufs"])
                        nc.tensor.matmul(ps_s[:, 0, oa:NB],
                                         KTp[h][:, a * P:(a + 1) * P],
                                         QTp[h][:, qsl0 + oa:qsl0 + NB],
                                         start=True, stop=True)
                        nc.tensor.matmul(ps_s[:, 1, oc:NB],
                                         KTp[h][:, c * P:(c + 1) * P],
                                         QTp[h][:, qsl0 + oc:qsl0 + NB],
                                         start=True, stop=True)
                        ex = apool.tile([P, 2, NB], BF16,
                                        name=f"ex_{b}_{h}_{qb}_{p}",
                                        tag="ex", bufs=o["ex_bufs"])
                        if oa == oc:
                            nc.scalar.activation(ex[:, :, oa:NB],
                                                 ps_s[:, :, oa:NB],
                                                 mybir.ActivationFunctionType.Exp,
                                                 scale=SCALE)
                        else:
                            nc.scalar.activation(ex[:, 0, oa:NB],
                                                 ps_s[:, 0, oa:NB],
                                                 mybir.ActivationFunctionType.Exp,
                                                 scale=SCALE)
                            nc.scalar.activation(ex[:, 1, oc:NB],
                                                 ps_s[:, 1, oc:NB],
                                                 mybir.ActivationFunctionType.Exp,
                                                 scale=SCALE)
                        return (p, ex, oa, oc)

                    def emit_post(st):
                        p, ex, oa, oc = st
                        a, c = 2 * p, 2 * p + 1
                        # zero dead prefixes of narrowed (diagonal) tiles
                        if oa > 0:
                            nc.gpsimd.memset(ex[:, 0, 0:oa], 0.0)
                        if oc > 0:
                            nc.gpsimd.memset(ex[:, 1, 0:oc], 0.0)
                        # triangle masks on diagonal tiles
                        for half, kt, off in ((0, a, oa), (1, c, oc)):
                            if kt >= 4 * qb:
                                sl = ex[:, half, off:off + P]
                                if o["mask_eng"] == "P":
                                    nc.gpsimd.tensor_mul(sl, sl, tri[:])
                                else:
                                    nc.vector.tensor_mul(sl, sl, tri[:])
                        # denominator partial: exs[p%2] += ex.lo + ex.hi
                        tpr = apool.tile([P, NB], BF16,
                                         name=f"tp_{b}_{h}_{qb}_{p}",
                                         tag="tpr", bufs=2)
                        nc.vector.tensor_add(tpr[:], ex[:, 0, :], ex[:, 1, :])
                        cp = o["chain_pat"][qb] if isinstance(o["chain_pat"], dict) else o["chain_pat"]
                        eng = getattr(nc, _ENG_MAP[cp[p % 2]])
                        if p < 2:
                            eng.tensor_copy(exs[p % 2][:], tpr[:])
                        else:
                            eng.tensor_add(exs[p % 2][:], exs[p % 2][:], tpr[:])
                        # AV accumulation
                        nc.tensor.matmul(ps_o[:, oa:NB],
                                         Vp[a][:, h * P:(h + 1) * P],
                                         ex[:, 0, oa:NB],
                                         start=(p == 0), stop=False,
                                         skip_group_check=True)
                        nc.tensor.matmul(ps_o[:, oc:NB],
                                         Vp[c][:, h * P:(h + 1) * P],
                                         ex[:, 1, oc:NB],
                                         start=False, stop=(p == npair - 1),
                                         skip_group_check=True)

                    # software pipeline: scores/exp of p+1 before AV of p
                    if o["pipe"]:
                        st = emit_scores_exp(0)
                        for p in range(npair):
                            nxt = (emit_scores_exp(p + 1)
                                   if p + 1 < npair else None)
                            emit_post(st)
                            st = nxt
                            yield
                    else:
                        for p in range(npair):
                            emit_post(emit_scores_exp(p))
                            yield
                    # denominator: merge chains, cross-partition reduce, recip
                    rec = apool.tile([1, NB], F32R, name=f"rec_{b}_{h}_{qb}",
                                     tag="rec", bufs=2)
                    dn = apool.tile([1, NB], FP32, name=f"dn_{b}_{h}_{qb}",
                                    tag="dn", bufs=2)
                    if o["chain_bf16"]:
                        mrg = apool.tile([P, NB], BF16, name=f"mg_{b}_{h}_{qb}",
                                         tag="mrg", bufs=2)
                        nc.vector.tensor_add(mrg[:], exs[0][:], exs[1][:])
                        nc.gpsimd.tensor_reduce(dn[:1, :], mrg[:],
                                                axis=mybir.AxisListType.C,
                                                op=mybir.AluOpType.add)
                    else:
                        dn1 = apool.tile([1, NB], FP32, name=f"dn1_{b}_{h}_{qb}",
                                         tag="dn1", bufs=2)
                        nc.gpsimd.tensor_reduce(dn[:1, :], exs[0][:],
                                                axis=mybir.AxisListType.C,
                                                op=mybir.AluOpType.add)
                        nc.gpsimd.tensor_reduce(dn1[:1, :], exs[1][:],
                                                axis=mybir.AxisListType.C,
                                                op=mybir.AluOpType.add)
                        nc.vector.tensor_add(dn[:1, :], dn[:1, :], dn1[:1, :])
                    with nc.allow_low_precision(reason="f32r recip: tf32 ok"):
                        nc.vector.reciprocal(rec[:1, :], dn[:1, :])

                    def close():
                        ps_bc = ps_tile("Y", [P, NB], f"psbc_{b}_{h}_{qb}",
                                        o["y_bufs"])
                        nc.tensor.matmul(ps_bc[:], onr[:1, :], rec[:1, :],
                                         start=True, stop=True)
                        bc = apool.tile([P, NB], FP32, name=f"bc_{b}_{h}_{qb}",
                                        tag="bc", bufs=2)
                        nc.vector.tensor_copy(bc[:], ps_bc[:])
                        nc.vector.tensor_mul(outTp[h][:, qsl0:qsl0 + NB],
                                             ps_o[:], bc[:])
                    return close

                # ============ proj units ============
                yts = {}

                def proj_unit(b, tt, eb, eng_code, narrow_dma=False,
                              tag="Y"):
                    ps_y = ps_tile(tag, [P, NB], f"psy_{b}_{tt}_{eb}",
                                   o["y_bufs"])
                    nc.tensor.matmul(ps_y[:], outTp[0][:, tt * P:(tt + 1) * P],
                                     wo_sb[0][:, eb * NB:(eb + 1) * NB],
                                     start=True, stop=False)
                    nc.tensor.matmul(ps_y[:], outTp[1][:, tt * P:(tt + 1) * P],
                                     wo_sb[1][:, eb * NB:(eb + 1) * NB],
                                     start=False, stop=True)
                    if eb == 0:
                        yts[(b, tt)] = apool.tile([P, QB, NB], BF16,
                                                  name=f"yt_{b}_{tt}",
                                                  tag="yt", bufs=o["yt_bufs"])
                    yt = yts[(b, tt)]
                    copy_on(eng_code, yt[:, eb, :], ps_y[:])
                    if narrow_dma:
                        # tail: stream each e-block out as soon as copied
                        nc.sync.dma_start(
                            y[b * T + tt * P:b * T + (tt + 1) * P,
                              eb * NB:(eb + 1) * NB], yt[:, eb, :])
                    elif eb == QB - 1:
                        nc.sync.dma_start(
                            y[b * T + tt * P:b * T + (tt + 1) * P, :], yt[:])

                def proj_thunks(b, pqb, pat, narrow_dma=False, tags="Y"):
                    th = []
                    i = 0
                    for tt in range(4 * pqb, 4 * pqb + 4):
                        for eb in range(QB):
                            code = pat[i % len(pat)]
                            tag = tags[i % len(tags)]
                            th.append(lambda b=b, tt=tt, eb=eb, code=code,
                                      tag=tag:
                                      proj_unit(b, tt, eb, code, narrow_dma,
                                                tag))
                            i += 1
                    return th

                # ============ weaving driver ============
                def weave(gen, fillers, carry, defer=None):
                    """Run gen; after each yield emit carry thunks (once,
                    after o['defer_pairs'] yields) and a fair share of
                    fillers (popped from the shared list)."""
                    n = 0
                    held = 0
                    try:
                        while True:
                            next(gen)
                            n += 1
                            if n >= o["defer_pairs"] and carry:
                                for fn in carry:
                                    fn()
                                carry = []
                            if defer:
                                held += defer.pop(0)
                            if not carry:
                                while held > 0 and fillers:
                                    fillers.pop(0)()
                                    held -= 1
                    except StopIteration as si:
                        for fn in carry:
                            fn()
                        return si.value

                def share(nfill, nsteps):
                    base, rem = divmod(nfill, nsteps)
                    return [base + (1 if i < rem else 0) for i in range(nsteps)]

                def attention_unit(b, qb, fillers, carry, reserve=3,
                                   final=False):
                    npair = 2 * qb + 2
                    # hold a few fillers back to cover the close-out chain
                    # latency after the last AV pair
                    nres = min(reserve, len(fillers))
                    sh = share(len(fillers) - nres, 2 * npair)
                    close0 = weave(attention_gen(b, 0, qb), fillers, carry,
                                   defer=sh[:npair])
                    close1 = weave(attention_gen(b, 1, qb), fillers, [close0],
                                   defer=sh[npair:])
                    if final:
                        for fn in fillers:
                            fn()
                        del fillers[:]
                        close1()
                        return []
                    for fn in fillers:  # reserved + leftovers
                        fn()
                    del fillers[:]
                    return [close1]

                def prefix(gen, n):
                    for _ in range(n):
                        next(gen)
                        yield

                # ============ main schedule ============
                carry = []
                xts = {}
                for b in range(B):
                    # ---- phase 1 (+ second half of prev batch qb3 proj) ----
                    if b > 0:
                        ph1_fill = proj_thunks(b - 1, 3, o["pat_p1"])[8:]
                        qb0_fill = proj_thunks(b - 1, 3, o["pat_attn"][3])[:8]
                    else:
                        ph1_fill, qb0_fill = [], []
                    if b == 0:
                        xts["cur"] = load_nb(0, 0)
                    for nb in range(QB - 1):
                        nxt = [b, nb + 1]
                        mid = (lambda nxt=nxt:
                               xts.__setitem__("next", load_nb(*nxt)))
                        weave(phase1_nb(b, nb, xts["cur"], mid), ph1_fill,
                              carry,
                              defer=share(2, KP + 1) if ph1_fill else None)
                        xts["cur"] = xts["next"]
                        carry = []
                    # nb3: QK sweep + V01; the V2/V3 tail becomes PE filler
                    # for the attention(qb0) unit
                    mid = ((lambda: xts.__setitem__("next", load_nb(b + 1, 0)))
                           if b + 1 < B else None)
                    g_ph = phase1_nb(b, 3, xts["cur"], mid)
                    weave(prefix(g_ph, 9), ph1_fill, carry,
                          defer=share(2, 10) if ph1_fill else None)
                    if b + 1 < B:
                        xts["cur"] = xts["next"]
                    carry = []
                    for fn in ph1_fill:
                        fn()
                    tails = [(lambda: next(g_ph, None)) for _ in range(10)]
                    # ---- qb0 unit merged with phase-1 V tail ----
                    # one V2 step first fixes the O-slot rotation so the V3
                    # sweep can safely cover the h0 close-out chain
                    tails.pop(0)()
                    close0 = weave(attention_gen(b, 0, 0), tails, carry,
                                   defer=[2, 2])
                    for _ in range(3):  # V3 progress covers the dn/recip chain
                        if tails:
                            tails.pop(0)()
                    close0()
                    fill2 = tails + qb0_fill
                    close1 = weave(attention_gen(b, 1, 0), fill2,
                                   [], defer=[2, 2])
                    for fn in fill2:
                        fn()
                    carry = [close1]
                    carry = attention_unit(
                        b, 1, proj_thunks(b, 0, o["pat_attn"][0]), carry)
                    carry = attention_unit(
                        b, 2, proj_thunks(b, 1, o["pat_attn"][1]), carry)
                    carry = attention_unit(
                        b, 3, proj_thunks(b, 2, o["pat_attn"][2]
                                          if b < B - 1
                                          else "DADADADADA" + "A" * 6), carry,
                        reserve=6, final=(b == B - 1))
                # ---- tail: close-out then final batch qb3 proj ----
                for c in carry:
                    c()
                carry = []
                for fn in proj_thunks(B - 1, 3, o["pat_tail"],
                                      narrow_dma=False, tags="YO"):
                    fn()
                if debug:
                    for h in range(HPC):
                        nc.sync.dma_start(dbg["QT"][h], QTp[h][:])
                        nc.sync.dma_start(dbg["KT"][h], KTp[h][:])
                        nc.sync.dma_start(dbg["outT"][h], outTp[h][:])
                    for kt in range(NT):
                        nc.sync.dma_start(dbg["V"][kt], Vp[kt][:])

    nc.compile()
    return nc


def prep_inputs(x, w_qkv, w_o):
    """Host-side shard prep. Returns per-core input maps (bf16)."""
    bf = mybir.dt.np(BF16)
    x = np.asarray(x, dtype=np.float32).reshape(B * T, D)
    xT = np.ascontiguousarray(x.T).reshape(KC, P, B * T).astype(bf)
    w_qkv = np.asarray(w_qkv, dtype=np.float32)
    w_o = np.asarray(w_o, dtype=np.float32)

    tri = np.zeros((P, P), dtype=np.float32)
    kp = np.arange(P)[:, None]
    qu = np.arange(P)[None, :]
    tri[kp <= qu] = 1.0
    tri = tri.astype(bf)
    onr = np.ones((1, P), dtype=np.float32)

    in_maps = []
    for c in range(NCORES):
        h0, h1 = HPC * c, HPC * c + 1
        cols = []
        for h in (h0, h1):
            cols += [w_qkv[h * DK:(h + 1) * DK],            # Q rows
                     w_qkv[D + h * DK:D + (h + 1) * DK]]    # K rows
        # reorder to Q0 K0 Q1 K1 then V0 V1
        cols = [cols[0], cols[1], cols[2], cols[3],
                w_qkv[2 * D + h0 * DK:2 * D + (h0 + 1) * DK],
                w_qkv[2 * D + h1 * DK:2 * D + (h1 + 1) * DK]]
        w = np.ascontiguousarray(
            np.concatenate(cols, 0).T).reshape(KC, P, WC).astype(bf)
        wo = np.ascontiguousarray(
            w_o[:, HPC * DK * c:HPC * DK * (c + 1)].T).astype(bf)
        in_maps.append({
            "xT": xT, "w": w, "woT": wo, "tri": tri, "onr": onr,
        })
    return in_maps


_nc_cache = {}


def get_nc(debug=False, **opts):
    key = (debug, tuple(sorted((k, str(v)) for k, v in opts.items())))
    if key not in _nc_cache:
        _nc_cache[key] = build(debug=debug, **opts)
    return _nc_cache[key]


def run(x, w_qkv, w_o, debug=False, **opts):
    nc = get_nc(debug=debug, **opts)
    in_maps = prep_inputs(x, w_qkv, w_o)
    res = bass_utils.run_bass_kernel_spmd(nc, in_maps, core_ids=list(range(NCORES)))
    return res


def kernel(x, w_qkv, w_o):
    res = run(x, w_qkv, w_o)
    y = res.results[0]["y"].astype(np.float64)
    for c in range(1, NCORES):
        y += res.results[c]["y"]
    return y.astype(np.float32).reshape(B, T, D)


# revision 42
# speedup vs baseline: 1.2826x; 1.0027x over previous
"""Trainium2 Bass kernel for causal multi-head attention (dense transformer block).

Math (reference semantics):
    qkv = x @ w_qkv.T ; split into Q,K,V heads [B,H,T,dk]
    (rotary in the reference rotates Q and K of head h by a constant,
     time-independent orthogonal rotation R_h; since scores = (R_h q)·(R_h k)
     = q·k, the rotation cancels exactly and is skipped here)
    scores = causal_mask(Q @ K.T / sqrt(dk)); attn = softmax(scores)
    out = attn @ V ; y = out @ w_o.T

Sharding: head-parallel over 8 cores (2 heads/core, both batches).  Each core
computes a partial y (its heads' contribution through w_o columns); the host
sums the 8 partials (the "all-reduce").

v3 design (vs the f32r baseline):
  * All matmul inputs bf16 (same 1 cyc/row PE rate as f32r at wide free dims,
    half the DMA + SBUF).  PSUM accumulation stays fp32.
  * Phase 1 is k-outer: 6 concurrent PSUM groups (Q/K for 2 heads in two
    2-bank "S" tiles, V written DIRECTLY in [token, dk] layout into "O"
    tiles) so PE starts as soon as the first weight/x chunk lands and no
    V^T->V transposes are needed.  PSUM drains on ACT/DVE (GpSimd cannot
    touch PSUM), chunk-PAIR DMAs halve HWDGE descriptor-queue pressure.
  * Softmax denominator: bf16 pair-add (DVE 4x mode) + fp32 running sums
    split into two chains (GpSimd + DVE), then two GpSimd cross-partition
    (axis=C) reduces - no [1,512] ones-matmuls on PE (saves ~34us PE).
  * Causal narrowing: diagonal key-tiles only compute the live q-suffix in
    scores/AV/exp; the dead ex prefix is zeroed by a GpSimd memset; the
    128x128 causal triangle is masked by a GpSimd multiply.
  * proj(qb) units are woven between the attention kt-pairs of the next
    unit (qb3 into the next batch's phase 1 + qb0 unit) so the exp-gated
    stretches of attention get PE filler; yt PSUM->SBUF copies alternate
    ACT/DVE; y stores go out as one wide DMA per token tile.
  * softmax close-out chains (reduce -> recip -> broadcast-mm -> normalize)
    are deferred into the following instruction stream so PE (in-order)
    never waits on them.
"""

import contextlib

import numpy as np

import concourse.bacc as bacc
import concourse.bass as bass
import concourse.mybir as mybir
import concourse.tile as tile
from concourse import bass_utils

B, T, D, H, DK = 2, 2048, 2048, 16, 128
NCORES = 8
HPC = H // NCORES  # heads per core
P = 128
NB = 512           # q-block / token-block / e-block width
KC = D // P        # 16 contraction chunks of the model dim
KP = KC // 2       # chunk pairs
QB = T // NB       # 4 q blocks per batch
NT = T // P        # 16 token tiles per batch
WC = 6 * P         # w columns per chunk: Q0 K0 Q1 K1 V0 V1
FP32 = mybir.dt.float32
F32R = mybir.dt.float32r
BF16 = mybir.dt.bfloat16
SCALE = 1.0 / np.sqrt(DK)

DEFAULT_OPTS = dict(
    ex_bufs=4, xt_bufs=2, yt_bufs=3, s_bufs=2, o_bufs=2, y_bufs=2,
    loop_n=1,
    # yt-copy engine cycle per *hosting location* of the proj units
    pat_attn={0: "DA", 1: "DA", 2: "DA", 3: "DA"},
    pat_p1="DA", pat_tail="AD",
    defer_pairs=2,
    chain_pat={0: "PD", 1: "PD", 2: "PD", 3: "DD"},  # per qb (even, odd pair)
    mask_eng="D",
    qkv_q="A", qkv_k="D", qkv_v="AD", pipe=True, chain_bf16=True,
)

_ENG_MAP = {"P": "gpsimd", "A": "scalar", "D": "vector"}


def _width(kt, qb):
    """Live q-suffix width of key tile kt within q-block qb (causal)."""
    j = kt - 4 * qb
    if j <= 0:
        return NB
    return NB - P * j


def build(debug=False, **opts):
    o = dict(DEFAULT_OPTS)
    o.update({k: v for k, v in opts.items() if k in DEFAULT_OPTS})
    nc = bacc.Bacc("TRN2", target_bir_lowering=False, debug=False,
                   num_devices=NCORES)
    # 3D dram layouts allow one DMA per chunk-pair / token tile
    xT = nc.dram_tensor("xT", [KC, P, B * T], BF16, kind="ExternalInput")
    w_d = nc.dram_tensor("w", [KC, P, WC], BF16, kind="ExternalInput")
    woT = nc.dram_tensor("woT", [HPC * DK, D], BF16, kind="ExternalInput")
    tri_d = nc.dram_tensor("tri", [P, P], BF16, kind="ExternalInput")
    onr_d = nc.dram_tensor("onr", [1, P], F32R, kind="ExternalInput")
    y = nc.dram_tensor("y", [B * T, D], BF16, kind="ExternalOutput")
    dbg = {}
    if debug:
        for nm in ("QT", "KT", "outT"):
            dbg[nm] = nc.dram_tensor(f"dbg_{nm}", [HPC, P, T], BF16,
                                     kind="ExternalOutput")
        dbg["V"] = nc.dram_tensor("dbg_V", [NT, P, 2 * P], BF16,
                                  kind="ExternalOutput")

    with tile.TileContext(nc) as tc:
        with (
            tc.tile_pool(name="const", bufs=1) as cpool,
            tc.tile_pool(name="xp", bufs=1) as xpool,
            tc.tile_pool(name="qkv", bufs=1) as qpool,
            tc.tile_pool(name="attn", bufs=1) as apool,
            tc.tile_pool(name="ps", bufs=1, space="PSUM") as pspool,
        ):
            # ---- constants / weights resident in SBUF ----
            w_sb = [cpool.tile([P, 2, WC], BF16, name=f"w_{kp}") for kp in range(KP)]
            wo_sb = [cpool.tile([P, D], BF16, name=f"wo_{h}") for h in range(HPC)]
            tri = cpool.tile([P, P], BF16, name="tri")
            onr = cpool.tile([1, P], F32R, name="onr")

            def wqk(k, m):  # m in 0..3 = Q0 K0 Q1 K1 of chunk k
                return w_sb[k // 2][:, k % 2, m * P:(m + 1) * P]

            def wvv(k):     # V columns (both heads) of chunk k
                return w_sb[k // 2][:, k % 2, 4 * P:6 * P]

            # persistent per-batch state (WAR deps recycle across batches)
            QTp = [qpool.tile([P, T], BF16, name=f"QT{h}") for h in range(HPC)]
            KTp = [qpool.tile([P, T], BF16, name=f"KT{h}") for h in range(HPC)]
            Vp = [qpool.tile([P, 2 * P], BF16, name=f"V{kt}") for kt in range(NT)]
            outTp = [qpool.tile([P, T], BF16, name=f"outT{h}") for h in range(HPC)]

            def ps_tile(tag, shape, name, bufs):
                return pspool.tile(shape, FP32, name=name, tag=tag, bufs=bufs)

            def copy_on(code, dst, src):
                eng = _ENG_MAP[code]
                if eng == "gpsimd":
                    nc.gpsimd.tensor_copy(dst, src)
                elif eng == "scalar":
                    nc.scalar.copy(dst, src)
                else:
                    nc.vector.tensor_copy(dst, src)

            loop_ctx = (tc.For_i(0, o["loop_n"], 1, hint_engines=(
                            mybir.EngineType.PE, mybir.EngineType.Activation,
                            mybir.EngineType.DVE, mybir.EngineType.SP,
                            mybir.EngineType.Pool))
                        if o["loop_n"] > 1 else contextlib.nullcontext())

            if o["loop_n"] > 1:
                # weights/constants loaded once, outside the HW loop
                for kp in range(KP):
                    nc.sync.dma_start(w_sb[kp][:], w_d[2 * kp:2 * kp + 2])
                nc.sync.dma_start(tri[:], tri_d[:, :])
                nc.sync.dma_start(onr[:], onr_d[:, :])
                for h in range(HPC):
                    nc.sync.dma_start(wo_sb[h][:], woT[h * P:(h + 1) * P, :])

            with loop_ctx:
                # ============ phase 1 generator (one token block) ============
                def load_nb(b, nb):
                    """Allocate + DMA the x tiles for token block (b, nb)."""
                    col0 = b * T + nb * NB
                    xt = [xpool.tile([P, 2, NB], BF16, name=f"x{kp}_{b}_{nb}",
                                     tag=f"x{kp}", bufs=o["xt_bufs"])
                          for kp in range(KP)]
                    first = b == 0 and nb == 0 and o["loop_n"] == 1
                    for kp in range(KP):
                        if first and kp == 0:
                            # column-split the first w pair-DMA (row interleave
                            # preserved) and slot x0 between the halves so the
                            # first Q/K matmuls unblock after w0a + x0
                            nc.sync.dma_start(w_sb[0][:, :, 0:2 * P],
                                              w_d[0:2, :, 0:2 * P])
                            nc.sync.dma_start(
                                xt[0][:], xT[0:2, :, col0:col0 + NB])
                            nc.sync.dma_start(w_sb[0][:, :, 2 * P:WC],
                                              w_d[0:2, :, 2 * P:WC])
                            continue
                        if first:
                            nc.sync.dma_start(w_sb[kp][:], w_d[2 * kp:2 * kp + 2])
                        nc.sync.dma_start(xt[kp][:],
                                          xT[2 * kp:2 * kp + 2, :, col0:col0 + NB])
                        if first and kp == 1:
                            nc.sync.dma_start(tri[:], tri_d[:, :])
                            nc.sync.dma_start(onr[:], onr_d[:, :])
                    return xt

                def phase1_nb(b, nb, xt, mid=None):
                    """QKV projection for token block (b, nb), k-outer.
                    Yields after each chunk-pair (8) + drains; `mid` thunk
                    (next-block prefetch) fires after chunk-pair 5."""
                    if b == 0 and nb == 1 and o["loop_n"] == 1:
                        for h in range(HPC):
                            nc.sync.dma_start(wo_sb[h][:], woT[h * P:(h + 1) * P, :])

                    S0 = ps_tile("S", [P, 2, NB], f"p1s0_{b}_{nb}", o["s_bufs"])
                    S1 = ps_tile("S", [P, 2, NB], f"p1s1_{b}_{nb}", o["s_bufs"])
                    # V token-tile groups need a PSUM bank each (one
                    # accumulation group per bank): two sub-sweeps of 2.
                    V01 = [ps_tile("O", [P, NB], f"p1v{t}_{b}_{nb}", o["o_bufs"])
                           for t in range(2)]
                    for kp in range(KP):
                        for half in range(2):
                            k = 2 * kp + half
                            st, sp = k == 0, k == KC - 1
                            xk = xt[kp][:, half, :]
                            nc.tensor.matmul(S0[:, 0, :], wqk(k, 0), xk,
                                             start=st, stop=sp)
                            nc.tensor.matmul(S0[:, 1, :], wqk(k, 1), xk,
                                             start=st, stop=sp)
                            nc.tensor.matmul(S1[:, 0, :], wqk(k, 2), xk,
                                             start=st, stop=sp)
                            nc.tensor.matmul(S1[:, 1, :], wqk(k, 3), xk,
                                             start=st, stop=sp)
                            for t in range(2):
                                nc.tensor.matmul(V01[t][:, 0:2 * P],
                                                 xt[kp][:, half, t * P:(t + 1) * P],
                                                 wvv(k), start=st, stop=sp)
                        if kp == 5 and mid is not None:
                            mid()
                        yield
                    # drain V first (V2/V3 sweeps wait on these PSUM
                    # banks), then QK (next block's S tiles are far off)
                    csl = slice(nb * NB, (nb + 1) * NB)
                    for t in range(2):
                        copy_on(o["qkv_v"][t % len(o["qkv_v"])],
                                Vp[nb * 4 + t][:], V01[t][:, 0:2 * P])
                    copy_on(o["qkv_q"], QTp[0][:, csl], S0[:, 0, :])
                    copy_on(o["qkv_k"], KTp[0][:, csl], S0[:, 1, :])
                    copy_on(o["qkv_q"], QTp[1][:, csl], S1[:, 0, :])
                    copy_on(o["qkv_k"], KTp[1][:, csl], S1[:, 1, :])
                    yield
                    # V2 then V3 sequentially: each holds only ONE O slot,
                    # so attention(qb0) can interleave using the other slot
                    for t in range(2, 4):
                        Vt = ps_tile("O", [P, NB], f"p1v{t}_{b}_{nb}",
                                     o["o_bufs"])
                        for kp in range(KP):
                            for half in range(2):
                                k = 2 * kp + half
                                nc.tensor.matmul(Vt[:, 0:2 * P],
                                                 xt[kp][:, half,
                                                        t * P:(t + 1) * P],
                                                 wvv(k), start=(k == 0),
                                                 stop=(k == KC - 1))
                            if kp % 2 == 1:
                                yield
                        copy_on(o["qkv_v"][t % len(o["qkv_v"])],
                                Vp[nb * 4 + t][:], Vt[:, 0:2 * P])
                        yield

                # ============ attention generator (one head) ============
                def attention_gen(b, h, qb):
                    """Yields once per kt-pair.  Returns the deferred
                    close-out thunk (bcmm + normalize)."""
                    nkt = 4 * qb + 4
                    qsl0 = qb * NB
                    ps_o = ps_tile("O", [P, NB], f"pso_{b}_{h}_{qb}", o["o_bufs"])
                    # two running-sum chains: even pairs / odd pairs
                    cdt = BF16 if o["chain_bf16"] else FP32
                    exs = [apool.tile([P, NB], cdt, name=f"exs{i}_{b}_{h}_{qb}",
                                      tag=f"exsum{i}", bufs=2) for i in range(2)]
                    npair = nkt // 2

                    def emit_scores_exp(p):
                        a, c = 2 * p, 2 * p + 1
                        oa, oc = NB - _width(a, qb), NB - _width(c, qb)
                        ps_s = ps_tile("S", [P, 2, NB], f"pss_{b}_{h}_{qb}_{p}",
                                       o["s_bufs"])
                        nc.tensor.matmul(ps_s[:, 0, oa:NB],
                                         KTp[h][:, a * P:(a + 1) * P],
                                         QTp[h][:, qsl0 + oa:qsl0 + NB],
                                         start=True, stop=True)
                        nc.tensor.matmul(ps_s[:, 1, oc:NB],
                                         KTp[h][:, c * P:(c + 1) * P],
                                         QTp[h][:, qsl0 + oc:qsl0 + NB],
                                         start=True, stop=True)
                        ex = apool.tile([P, 2, NB], BF16,
                                        name=f"ex_{b}_{h}_{qb}_{p}",
                                        tag="ex", bufs=o["ex_bufs"])
                        if oa == oc:
                            nc.scalar.activation(ex[:, :, oa:NB],
                                                 ps_s[:, :, oa:NB],
                                                 mybir.ActivationFunctionType.Exp,
                                                 scale=SCALE)
                        else:
                            nc.scalar.activation(ex[:, 0, oa:NB],
                                                 ps_s[:, 0, oa:NB],
                                                 mybir.ActivationFunctionType.Exp,
                                                 scale=SCALE)
                            nc.scalar.activation(ex[:, 1, oc:NB],
                                                 ps_s[:, 1, oc:NB],
                                                 mybir.ActivationFunctionType.Exp,
                                                 scale=SCALE)
                        return (p, ex, oa, oc)

                    def emit_post(st):
                        p, ex, oa, oc = st
                        a, c = 2 * p, 2 * p + 1
                        # zero dead prefixes of narrowed (diagonal) tiles
                        if oa > 0:
                            nc.gpsimd.memset(ex[:, 0, 0:oa], 0.0)
                        if oc > 0:
                            nc.gpsimd.memset(ex[:, 1, 0:oc], 0.0)
                        # triangle masks on diagonal tiles
                        for half, kt, off in ((0, a, oa), (1, c, oc)):
                            if kt >= 4 * qb:
                                sl = ex[:, half, off:off + P]
                                if o["mask_eng"] == "P":
                                    nc.gpsimd.tensor_mul(sl, sl, tri[:])
                                else:
                                    nc.vector.tensor_mul(sl, sl, tri[:])
                        # denominator partial: exs[p%2] += ex.lo + ex.hi
                        tpr = apool.tile([P, NB], BF16,
                                         name=f"tp_{b}_{h}_{qb}_{p}",
                                         tag="tpr", bufs=2)
                        nc.vector.tensor_add(tpr[:], ex[:, 0, :], ex[:, 1, :])
                        cp = o["chain_pat"][qb] if isinstance(o["chain_pat"], dict) else o["chain_pat"]
                        eng = getattr(nc, _ENG_MAP[cp[p % 2]])
                        if p < 2:
                            eng.tensor_copy(exs[p % 2][:], tpr[:])
                        else:
                            eng.tensor_add(exs[p % 2][:], exs[p % 2][:], tpr[:])
                        # AV accumulation
                        nc.tensor.matmul(ps_o[:, oa:NB],
                                         Vp[a][:, h * P:(h + 1) * P],
                                         ex[:, 0, oa:NB],
                                         start=(p == 0), stop=False,
                                         skip_group_check=True)
                        nc.tensor.matmul(ps_o[:, oc:NB],
                                         Vp[c][:, h * P:(h + 1) * P],
                                         ex[:, 1, oc:NB],
                                         start=False, stop=(p == npair - 1),
                                         skip_group_check=True)

                    # software pipeline: scores/exp of p+1 before AV of p
                    if o["pipe"]:
                        st = emit_scores_exp(0)
                        for p in range(npair):
                            nxt = (emit_scores_exp(p + 1)
                                   if p + 1 < npair else None)
                            emit_post(st)
                            st = nxt
                            yield
                    else:
                        for p in range(npair):
                            emit_post(emit_scores_exp(p))
                            yield
                    # denominator: merge chains, cross-partition reduce, recip
                    rec = apool.tile([1, NB], F32R, name=f"rec_{b}_{h}_{qb}",
                                     tag="rec", bufs=2)
                    dn = apool.tile([1, NB], FP32, name=f"dn_{b}_{h}_{qb}",
                                    tag="dn", bufs=2)
                    if o["chain_bf16"]:
                        mrg = apool.tile([P, NB], BF16, name=f"mg_{b}_{h}_{qb}",
                                         tag="mrg", bufs=2)
                        nc.vector.tensor_add(mrg[:], exs[0][:], exs[1][:])
                        nc.gpsimd.tensor_reduce(dn[:1, :], mrg[:],
                                                axis=mybir.AxisListType.C,
                                                op=mybir.AluOpType.add)
                    else:
                        dn1 = apool.tile([1, NB], FP32, name=f"dn1_{b}_{h}_{qb}",
                                         tag="dn1", bufs=2)
                        nc.gpsimd.tensor_reduce(dn[:1, :], exs[0][:],
                                                axis=mybir.AxisListType.C,
                                                op=mybir.AluOpType.add)
                        nc.gpsimd.tensor_reduce(dn1[:1, :], exs[1][:],
                                                axis=mybir.AxisListType.C,
                                                op=mybir.AluOpType.add)
                        nc.vector.tensor_add(dn[:1, :], dn[:1, :], dn1[:1, :])
                    with nc.allow_low_precision(reason="f32r recip: tf32 ok"):
                        nc.vector.reciprocal(rec[:1, :], dn[:1, :])

                    def close():
                        ps_bc = ps_tile("Y", [P, NB], f"psbc_{b}_{h}_{qb}",
                                        o["y_bufs"])
                        nc.tensor.matmul(ps_bc[:], onr[:1, :], rec[:1, :],
                                         start=True, stop=True)
                        bc = apool.tile([P, NB], FP32, name=f"bc_{b}_{h}_{qb}",
                                        tag="bc", bufs=2)
                        nc.vector.tensor_copy(bc[:], ps_bc[:])
                        nc.vector.tensor_mul(outTp[h][:, qsl0:qsl0 + NB],
                                             ps_o[:], bc[:])
                    return close

                # ============ proj units ============
                yts = {}

                def proj_unit(b, tt, eb, eng_code, narrow_dma=False,
                              tag="Y"):
                    ps_y = ps_tile(tag, [P, NB], f"psy_{b}_{tt}_{eb}",
                                   o["y_bufs"])
                    nc.tensor.matmul(ps_y[:], outTp[0][:, tt * P:(tt + 1) * P],
                                     wo_sb[0][:, eb * NB:(eb + 1) * NB],
                                     start=True, stop=False)
                    nc.tensor.matmul(ps_y[:], outTp[1][:, tt * P:(tt + 1) * P],
                                     wo_sb[1][:, eb * NB:(eb + 1) * NB],
                                     start=False, stop=True)
                    if eb == 0:
                        yts[(b, tt)] = apool.tile([P, QB, NB], BF16,
                                                  name=f"yt_{b}_{tt}",
                                                  tag="yt", bufs=o["yt_bufs"])
                    yt = yts[(b, tt)]
                    copy_on(eng_code, yt[:, eb, :], ps_y[:])
                    if narrow_dma:
                        # tail: stream each e-block out as soon as copied
                        nc.sync.dma_start(
                            y[b * T + tt * P:b * T + (tt + 1) * P,
                              eb * NB:(eb + 1) * NB], yt[:, eb, :])
                    elif eb == QB - 1:
                        nc.sync.dma_start(
                            y[b * T + tt * P:b * T + (tt + 1) * P, :], yt[:])

                def proj_thunks(b, pqb, pat, narrow_dma=False, tags="Y"):
                    th = []
                    i = 0
                    for tt in range(4 * pqb, 4 * pqb + 4):
                        for eb in range(QB):
                            code = pat[i % len(pat)]
                            tag = tags[i % len(tags)]
                            th.append(lambda b=b, tt=tt, eb=eb, code=code,
                                      tag=tag:
                                      proj_unit(b, tt, eb, code, narrow_dma,
                                                tag))
                            i += 1
                    return th

                # ============ weaving driver ============
                def weave(gen, fillers, carry, defer=None):
                    """Run gen; after each yield emit carry thunks (once,
                    after o['defer_pairs'] yields) and a fair share of
                    fillers (popped from the shared list)."""
                    n = 0
                    held = 0
                    try:
                        while True:
                            next(gen)
                            n += 1
                            if n >= o["defer_pairs"] and carry:
                                for fn in carry:
                                    fn()
                                carry = []
                            if defer:
                                held += defer.pop(0)
                            if not carry:
                                while held > 0 and fillers:
                                    fillers.pop(0)()
                                    held -= 1
                    except StopIteration as si:
                        for fn in carry:
                            fn()
                        return si.value

                def share(nfill, nsteps):
                    base, rem = divmod(nfill, nsteps)
                    return [base + (1 if i < rem else 0) for i in range(nsteps)]

                def attention_unit(b, qb, fillers, carry, reserve=3,
                                   final=False):
                    npair = 2 * qb + 2
                    # hold a few fillers back to cover the close-out chain
                    # latency after the last AV pair
                    nres = min(reserve, len(fillers))
                    sh = share(len(fillers) - nres, 2 * npair)
                    close0 = weave(attention_gen(b, 0, qb), fillers, carry,
                                   defer=sh[:npair])
                    close1 = weave(attention_gen(b, 1, qb), fillers, [close0],
                                   defer=sh[npair:])
                    if final:
                        for fn in fillers:
                            fn()
                        del fillers[:]
                        close1()
                        return []
                    for fn in fillers:  # reserved + leftovers
                        fn()
                    del fillers[:]
                    return [close1]

                def prefix(gen, n):
                    for _ in range(n):
                        next(gen)
                        yield

                # ============ main schedule ============
                carry = []
                xts = {}
                for b in range(B):
                    # ---- phase 1 (+ second half of prev batch qb3 proj) ----
                    if b > 0:
                        ph1_fill = proj_thunks(b - 1, 3, o["pat_p1"])[8:]
                        qb0_fill = proj_thunks(b - 1, 3, o["pat_attn"][3])[:8]
                    else:
                        ph1_fill, qb0_fill = [], []
                    if b == 0:
                        xts["cur"] = load_nb(0, 0)
                    for nb in range(QB - 1):
                        nxt = [b, nb + 1]
                        mid = (lambda nxt=nxt:
                               xts.__setitem__("next", load_nb(*nxt)))
                        weave(phase1_nb(b, nb, xts["cur"], mid), ph1_fill,
                              carry,
                              defer=share(2, KP + 1) if ph1_fill else None)
                        xts["cur"] = xts["next"]
                        carry = []
                    # nb3: QK sweep + V01; the V2/V3 tail becomes PE filler
                    # for the attention(qb0) unit
                    mid = ((lambda: xts.__setitem__("next", load_nb(b + 1, 0)))
                           if b + 1 < B else None)
                    g_ph = phase1_nb(b, 3, xts["cur"], mid)
                    weave(prefix(g_ph, 9), ph1_fill, carry,
                          defer=share(2, 10) if ph1_fill else None)
                    if b + 1 < B:
                        xts["cur"] = xts["next"]
                    carry = []
                    for fn in ph1_fill:
                        fn()
                    tails = [(lambda: next(g_ph, None)) for _ in range(10)]
                    # ---- qb0 unit merged with phase-1 V tail ----
                    # one V2 step first fixes the O-slot rotation so the V3
                    # sweep can safely cover the h0 close-out chain
                    tails.pop(0)()
                    close0 = weave(attention_gen(b, 0, 0), tails, carry,
                                   defer=[2, 2])
                    for _ in range(3):  # V3 progress covers the dn/recip chain
                        if tails:
                            tails.pop(0)()
                    close0()
                    fill2 = tails + qb0_fill
                    close1 = weave(attention_gen(b, 1, 0), fill2,
                                   [], defer=[2, 2])
                    for fn in fill2:
                        fn()
                    carry = [close1]
                    carry = attention_unit(
                        b, 1, proj_thunks(b, 0, o["pat_attn"][0]), carry)
                    carry = attention_unit(
                        b, 2, proj_thunks(b, 1, o["pat_attn"][1]), carry)
                    carry = attention_unit(
                        b, 3, proj_thunks(b, 2, o["pat_attn"][2]
                                          if b < B - 1
                                          else "DADADADADA" + "A" * 6), carry,
                        reserve=6, final=(b == B - 1))
                # ---- tail: close-out then final batch qb3 proj ----
                for c in carry:
                    c()
                carry = []
                for fn in proj_thunks(B - 1, 3, o["pat_tail"],
                                      narrow_dma=False, tags="YO"):
                    fn()
                if debug:
                    for h in range(HPC):
                        nc.sync.dma_start(dbg["QT"][h], QTp[h][:])
                        nc.sync.dma_start(dbg["KT"][h], KTp[h][:])
                        nc.sync.dma_start(dbg["outT"][h], outTp[h][:])
                    for kt in range(NT):
                        nc.sync.dma_start(dbg["V"][kt], Vp[kt][:])

    nc.compile()
    return nc


def prep_inputs(x, w_qkv, w_o):
    """Host-side shard prep. Returns per-core input maps (bf16)."""
    bf = mybir.dt.np(BF16)
    x = np.asarray(x, dtype=np.float32).reshape(B * T, D)
    xT = np.ascontiguousarray(x.T).reshape(KC, P, B * T).astype(bf)
    w_qkv = np.asarray(w_qkv, dtype=np.float32)
    w_o = np.asarray(w_o, dtype=np.float32)

    tri = np.zeros((P, P), dtype=np.float32)
    kp = np.arange(P)[:, None]
    qu = np.arange(P)[None, :]
    tri[kp <= qu] = 1.0
    tri = tri.astype(bf)
    onr = np.ones((1, P), dtype=np.float32)

    in_maps = []
    for c in range(NCORES):
        h0, h1 = HPC * c, HPC * c + 1
        cols = []
        for h in (h0, h1):
            cols += [w_qkv[h * DK:(h + 1) * DK],            # Q rows
                     w_qkv[D + h * DK:D + (h + 1) * DK]]    # K rows
        # reorder to Q0 K0 Q1 K1 then V0 V1
        cols = [cols[0], cols[1], cols[2], cols[3],
                w_qkv[2 * D + h0 * DK:2 * D + (h0 + 1) * DK],
                w_qkv[2 * D + h1 * DK:2 * D + (h1 + 1) * DK]]
        w = np.ascontiguousarray(
            np.concatenate(cols, 0).T).reshape(KC, P, WC).astype(bf)
        wo = np.ascontiguousarray(
            w_o[:, HPC * DK * c:HPC * DK * (c + 1)].T).astype(bf)
        in_maps.append({
            "xT": xT, "w": w, "woT": wo, "tri": tri, "onr": onr,
        })
    return in_maps


_nc_cache = {}


def get_nc(debug=False, **opts):
    key = (debug, tuple(sorted((k, str(v)) for k, v in opts.items())))
    if key not in _nc_cache:
        _nc_cache[key] = build(debug=debug, **opts)
    return _nc_cache[key]


def run(x, w_qkv, w_o, debug=False, **opts):
    nc = get_nc(debug=debug, **opts)
    in_maps = prep_inputs(x, w_qkv, w_o)
    res = bass_utils.run_bass_kernel_spmd(nc, in_maps, core_ids=list(range(NCORES)))
    return res


def kernel(x, w_qkv, w_o):
    res = run(x, w_qkv, w_o)
    y = res.results[0]["y"].astype(np.float64)
    for c in range(1, NCORES):
        y += res.results[c]["y"]
    return y.astype(np.float32).reshape(B, T, D)
